# revision 23
# baseline (speedup 1.0000x reference)
"""EdgeCNN (DGCNN) Bass/Tile kernel for TRN2 — one batch element per core.

Per edge-conv layer (N=1024 points, K=20 neighbors):
  1. PE: augmented matmul key[n,j] = 2<xn,xj> - S[j]  (ones-row trick folds
     the -S[j] column term; the -S[n] row term is rank-invariant and dropped)
  2. ACT: evict keys PSUM -> SBUF
  3. GPSIMD: AND low-10 mantissa bits, OR in column index j -> packed keys
  4. DVE: 3x max8 + 2x match_replace -> top-20 packed keys; extract j
  5. idx -> DRAM -> read back wrapped (partition = i%16); dma_gather of
     a = x @ (g~ Wn)^T rows SPLIT 4-WAY across SWDGE queues 0-3 (concurrent
     Q7 cpu pairs + DMA paths)
  6. DVE: strided reduce_max over k; PE: transpose(m) + c-matmul accumulate
  7. ACT: leaky-relu (Prelu alpha=0.2) PSUM -> next layer xT
Head: conv5 via K-chunk accumulation, global max-pool, 3 FC layers on PE.
"""

import contextlib

import numpy as np

import concourse.bass as bass
import concourse.bacc as bacc
import concourse.mybir as mybir
from concourse.tile import TileContext
from concourse.masks import make_identity

F32 = mybir.dt.float32
U32 = mybir.dt.uint32
I16 = mybir.dt.int16
F16 = mybir.dt.float16
F32R = mybir.dt.float32r


def _r(ap):
    return ap.bitcast(F32R)
AF = mybir.ActivationFunctionType
ALU = mybir.AluOpType
AX = mybir.AxisListType

N = 1024
KNN = 20
NT = 8
NQ = 4            # SWDGE queues
NEG_SLOPE = 0.2
BNI = np.float32(1.0 / np.sqrt(1.0 + 1e-5))
LAYERS = [(3, 64), (64, 64), (64, 128), (128, 256)]
NEG_BIG = -3.0e38


def host_prep(inp):
    """Fold BN scale/bias into weights; transpose for device layout."""
    d = {}
    for li, (C, O) in enumerate(LAYERS, start=1):
        W = inp[f'W{li}'].astype(np.float32)
        g = inp[f'g{li}'].astype(np.float32)
        b = inp[f'b{li}'].astype(np.float32)
        gt = g * BNI
        Wn = W[:, :C]
        Wc = W[:, C:]
        d[f'wnt{li}'] = np.ascontiguousarray((gt[:, None] * Wn).T)          # (C, O)
        d[f'wdt{li}'] = np.ascontiguousarray((gt[:, None] * (Wc - Wn)).T)   # (C, O)
        d[f'bs{li}'] = np.ascontiguousarray(b.reshape(max(1, O // 128), min(O, 128)).T)
    g5 = inp['g5'].astype(np.float32) * BNI
    d['w5t'] = np.ascontiguousarray((g5[:, None] * inp['W5']).T)            # (512, 512)
    d['b5'] = inp['b5'].reshape(1, 512).astype(np.float32).copy()
    g1 = inp['bng1'].astype(np.float32) * BNI
    d['wfc1'] = np.ascontiguousarray((g1[:, None] * inp['fc1_w']).T)        # (512, 256)
    bf1 = g1 * inp['fc1_b'].astype(np.float32) + inp['bnb1'].astype(np.float32)
    d['bfc1'] = np.ascontiguousarray(bf1.reshape(2, 128).T)                 # (128, 2)
    g2 = inp['bng2'].astype(np.float32) * BNI
    d['wfc2'] = np.ascontiguousarray((g2[:, None] * inp['fc2_w']).T)        # (256, 128)
    bf2 = g2 * inp['fc2_b'].astype(np.float32) + inp['bnb2'].astype(np.float32)
    d['bfc2'] = np.ascontiguousarray(bf2.reshape(128, 1))                   # (128, 1)
    d['wfc3'] = np.ascontiguousarray(inp['fc3_w'].T)                        # (128, 40)
    d['bfc3'] = inp['fc3_b'].reshape(1, 40).astype(np.float32).copy()
    return d


def build_nc():
    nc = bacc.Bacc("TRN2", target_bir_lowering=False, debug=False, num_devices=8,
                   num_swdge_queues=NQ)
    with TileContext(nc) as tc:
        _trace(nc, tc)
    nc.compile()
    return nc


def _trace(nc, tc):
    with contextlib.ExitStack() as ctx:
        dram = ctx.enter_context(tc.tile_pool(name="dram", bufs=1, space="DRAM"))
        consts = ctx.enter_context(tc.tile_pool(name="consts", bufs=1))
        persist = ctx.enter_context(tc.tile_pool(name="persist", bufs=1))
        sb = ctx.enter_context(tc.tile_pool(name="sb", bufs=2))
        kpcp = ctx.enter_context(tc.tile_pool(name="kpcp", bufs=2))
        keyp = ctx.enter_context(tc.tile_pool(name="keyp", bufs=2))
        smalls = ctx.enter_context(tc.tile_pool(name="smalls", bufs=4))
        idqp = ctx.enter_context(tc.tile_pool(name="idqp", bufs=4))
        gath = ctx.enter_context(tc.tile_pool(name="gath", bufs=3))
        mp = ctx.enter_context(tc.tile_pool(name="mp", bufs=1))
        psb = ctx.enter_context(tc.tile_pool(name="psb", bufs=1, space="PSUM"))
        pss = ctx.enter_context(tc.tile_pool(name="pss", bufs=2, space="PSUM"))

        # ---- DRAM I/O ----
        x_d = dram.tile([N, 3], F32, kind="ExternalInput", uniquify=False, name="x")
        win = {}
        for li, (C, O) in enumerate(LAYERS, start=1):
            win[f'wnt{li}'] = dram.tile([C, O], F32, kind="ExternalInput", uniquify=False, name=f"wnt{li}")
            win[f'wdt{li}'] = dram.tile([C, O], F32, kind="ExternalInput", uniquify=False, name=f"wdt{li}")
            win[f'bs{li}'] = dram.tile([min(O, 128), max(1, O // 128)], F32, kind="ExternalInput", uniquify=False, name=f"bs{li}")
        w5t_d = dram.tile([512, 512], F32, kind="ExternalInput", uniquify=False, name="w5t")
        b5_d = dram.tile([1, 512], F32, kind="ExternalInput", uniquify=False, name="b5")
        wfc1_d = dram.tile([512, 256], F32, kind="ExternalInput", uniquify=False, name="wfc1")
        bfc1_d = dram.tile([128, 2], F32, kind="ExternalInput", uniquify=False, name="bfc1")
        wfc2_d = dram.tile([256, 128], F32, kind="ExternalInput", uniquify=False, name="wfc2")
        bfc2_d = dram.tile([128, 1], F32, kind="ExternalInput", uniquify=False, name="bfc2")
        wfc3_d = dram.tile([128, 40], F32, kind="ExternalInput", uniquify=False, name="wfc3")
        bfc3_d = dram.tile([1, 40], F32, kind="ExternalInput", uniquify=False, name="bfc3")
        out_d = dram.tile([40, 1], F32, kind="ExternalOutput", uniquify=False, name="out")

        gdts = {1: F32, 2: F32, 3: F16, 4: F16}
        a_ds = {li: dram.tile([N, O], gdts[li], name=f"a_d{li}")
                for li, (C, O) in enumerate(LAYERS, start=1)}
        jw_ds = {li: dram.tile([N * KNN // 16, 128], I16, name=f"jw_d{li}")
                 for li in range(1, 5)}

        # ---- consts ----
        iotaJ = consts.tile([128, N], U32, tag="iotaJ")
        nc.gpsimd.iota(iotaJ[:, :], [[1, N]], base=0, channel_multiplier=0)
        ident = consts.tile([128, 128], F32, tag="ident")
        make_identity(nc, ident[:, :])
        onescol = consts.tile([128, 1], F32, tag="onescol")
        nc.vector.memset(onescol[:, :], 1.0)
        onescolR = consts.tile([128, 1], F32, tag="onescolR")
        nc.sync.dma_start(_r(onescolR[:, :]), _r(onescol[:, :]))
        onesrow = consts.tile([1, N], F32, tag="onesrow")
        nc.vector.memset(onesrow[:, :], 1.0)
        onesrowR = consts.tile([1, N], F32, tag="onesrowR")
        nc.sync.dma_start(_r(onesrowR[:, :]), _r(onesrow[:, :]))

        # persistent feature tensors (augmented with a trailing ones row
        # where the next layer uses the ones-trick, i.e. C_next + 1 <= 128)
        x0T = persist.tile([4, N], F32, tag="x0T")
        x1T = persist.tile([65, N], F32, tag="x1T")
        x2T = persist.tile([65, N], F32, tag="x2T")
        x3T = persist.tile([128, N], F32, tag="x3T")
        x4Ta = persist.tile([128, N], F32, tag="x4Ta")
        x4Tb = persist.tile([128, N], F32, tag="x4Tb")
        nc.sync.dma_start(_r(x0T[3:4, :]), _r(onesrow[0:1, :]))
        nc.sync.dma_start(_r(x1T[64:65, :]), _r(onesrow[0:1, :]))
        nc.sync.dma_start(_r(x2T[64:65, :]), _r(onesrow[0:1, :]))

        # load x transposed: x_d is (N, 3) row-major
        xap = x_d[:, :]
        nc.sync.dma_start(
            _r(x0T[0:3, :]),
            bass.AP(xap.tensor, xap.offset, [[1, 3], [3, N]]).bitcast(F32R))

        def edge_layer(li, xT, C, O, out_parts, post_tiles=None,
                       post_half=None):
            """xT: [C(+1), N] features (row C = ones iff aug). out_parts:
            list of (dest_ap, orow, ocol0)."""
            aug = (C + 1 <= 128) and li < 4
            gdt = gdts[li]
            a_d = a_ds[li]
            jw_d = jw_ds[li]
            jwap = jw_d[:, :]
            wnt = sb.tile([C, O], F32, tag="wnt")
            wdt = sb.tile([C, O], F32, tag="wdt")
            bs = sb.tile([min(O, 128), max(1, O // 128)], F32, tag="bs")
            nc.sync.dma_start(_r(wnt[:, :]), _r(win[f'wnt{li}'][:, :]))
            nc.sync.dma_start(_r(wdt[:, :]), _r(win[f'wdt{li}'][:, :]))
            nc.sync.dma_start(bs[:, :], win[f'bs{li}'][:, :])

            xsq = sb.tile([C, N], F32, tag="xsq")
            if aug:
                x2dA = sb.tile([C + 1, N], F32, tag="x2dA")
            else:
                x2dA = sb.tile([C, N], F32, tag="x2dA")
            # compute engines may only start at partition 0/32/64/96: stage
            # negS in a [1, N] tile and DMA into the augmented row otherwise
            direct = aug and C % 32 == 0
            negS = None if direct else sb.tile([1, N], F32, tag="negS")
            for h in range(2):
                cols = slice(h * 512, (h + 1) * 512)
                nc.scalar.activation(_r(xsq[:, cols]), xT[0:C, cols], AF.Square)
                nc.scalar.activation(
                    _r(x2dA[0:C, cols]), xT[0:C, cols], AF.Copy, bias=0.0,
                    scale=2.0)
                S_ps = pss.tile([1, 512], F32, tag="a")
                nc.tensor.matmul(
                    S_ps[:, :], lhsT=_r(onescolR[0:C, :]), rhs=_r(xsq[:, cols]),
                    start=True, stop=True, skip_group_check=True)
                dst = x2dA[C:C + 1, cols] if direct else negS[0:1, cols]
                nc.scalar.activation(_r(dst), S_ps[:, :], AF.Copy, bias=0.0,
                                     scale=-1.0)
                if aug and not direct:
                    nc.sync.dma_start(_r(x2dA[C:C + 1, cols]), _r(negS[0:1, cols]))

            negSc = sb.tile([128, NT], F32, tag="negSc")

            def emit_negSc(t):
                nsp = pss.tile([128, 1], F32, tag="a")
                nc.tensor.matmul(
                    nsp[:, :], lhsT=xsq[:, t * 128:(t + 1) * 128],
                    rhs=onescol[0:C, :],
                    start=True, stop=True, skip_group_check=True)
                nc.scalar.activation(
                    negSc[:, t:t + 1], nsp[:, :], AF.Copy, bias=0.0, scale=-1.0)

            # px PSUM tiles; c-part matmuls issued early (start=True)
            pxs = []
            for (dst_ap, orow, oc0) in out_parts:
                px = psb.tile([orow, N], F32, tag=f"px{oc0}")
                pxs.append(px)

            m = mp.tile([128, NT, O], F32, tag=f"m{li}")

            state = {}

            def emit_a_rows():
                # block a-tiles into [128, 512]-col PSUM groups: one eviction
                # and one (3-level AP) DMA per group instead of per tile
                tpg = 512 // O                      # tiles per group
                for g0 in range(0, NT, tpg):
                    ng = min(tpg, NT - g0)
                    a_ps = pss.tile([128, ng, O], F32, tag="a")
                    for ti in range(ng):
                        lt = xT[0:C, (g0 + ti) * 128:(g0 + ti + 1) * 128]
                        wv = wnt[:, :]
                        if O >= 256:
                            lt, wv = _r(lt), _r(wv)
                        nc.tensor.matmul(
                            a_ps[:, ti, :], lhsT=lt, rhs=wv,
                            start=True, stop=True, skip_group_check=True)
                    a_sb = sb.tile([128, ng, O], gdt, tag="a_sb")
                    nc.scalar.activation(
                        a_sb[:, :, :].rearrange("p t o -> p (t o)"),
                        a_ps[:, :, :].rearrange("p t o -> p (t o)"), AF.Copy)
                    adap = a_d[:, :]
                    dst = bass.AP(adap.tensor, adap.offset + g0 * 128 * O,
                                  [[O, 128], [128 * O, ng], [1, O]])
                    nc.sync.dma_start(dst, a_sb[:, :, :])

            def emit_c_parts():
                for pi, (dst_t, orow, oc0) in enumerate(out_parts):
                    for h in range(2):
                        cols = slice(h * 512, (h + 1) * 512)
                        nc.tensor.matmul(
                            pxs[pi][:, cols],
                            lhsT=_r(wdt[:, oc0:oc0 + orow]), rhs=_r(xT[0:C, cols]),
                            start=True, stop=False, skip_group_check=True)

            def emit_kp(t):
                tcols = slice(t * 128, (t + 1) * 128)
                kp = psb.tile([128, N], F32, tag="kp")
                for h in range(2):
                    cols = slice(h * 512, (h + 1) * 512)
                    if aug:
                        nc.tensor.matmul(
                            kp[:, cols], lhsT=_r(xT[:, tcols]), rhs=_r(x2dA[:, cols]),
                            start=True, stop=True, skip_group_check=True)
                    else:
                        nc.tensor.matmul(
                            kp[:, cols], lhsT=_r(xT[0:C, tcols]), rhs=_r(x2dA[:, cols]),
                            start=True, stop=False, skip_group_check=True)
                        nc.tensor.matmul(
                            kp[:, cols], lhsT=_r(onesrowR[0:1, tcols]),
                            rhs=_r(negS[0:1, cols]),
                            start=False, stop=True, skip_group_check=True)
                kpc = kpcp.tile([128, N], F32, tag="kpc")
                nc.scalar.activation(
                    kpc[:, :], kp[:, :], AF.Prelu, bias=negSc[:, t:t + 1],
                    scale=1.0, alpha=1.0)
                kb = keyp.tile([128, N], U32, tag="kb")
                nc.vector.tensor_scalar(
                    kb[:, :], kpc[:, :].bitcast(U32), 0xFFFFFC00, None,
                    op0=ALU.bitwise_and)
                nc.vector.tensor_tensor(
                    out=kb[:, :], in0=kb[:, :], in1=iotaJ[:, :], op=ALU.bitwise_or)
                state[t] = {'kb': kb}

            def emit_topk(t):
                kbf = state[t]['kb'][:, :].bitcast(F32)
                v24 = smalls.tile([128, 24], F32, tag="v24")
                nc.vector.max(v24[:, 0:8], kbf)
                nc.vector.match_replace(kbf, v24[:, 0:8], kbf, NEG_BIG)
                nc.vector.max(v24[:, 8:16], kbf)
                nc.vector.match_replace(kbf, v24[:, 8:16], kbf, NEG_BIG)
                nc.vector.max(v24[:, 16:24], kbf)
                j20 = smalls.tile([128, KNN], U32, tag="j20")
                nc.vector.tensor_scalar(
                    j20[:, :], v24[:, 0:KNN].bitcast(U32), 0x3FF, None,
                    op0=ALU.bitwise_and)
                jf20 = smalls.tile([128, KNN], F32, tag="jf20")
                nc.vector.tensor_copy(jf20[:, :], j20[:, :])
                state[t]['jf20'] = jf20

            def emit_idq(t):
                jT_ps = pss.tile([KNN, 128], F32, tag="a")
                nc.tensor.matmul(
                    jT_ps[:, :], lhsT=state[t]['jf20'][:, :], rhs=ident[:, 0:128],
                    is_transpose=True, start=True, stop=True, skip_group_check=True)
                jTi = smalls.tile([KNN, 128], I16, tag="jTi")
                nc.scalar.activation(jTi[:, :], jT_ps[:, :], AF.Copy)
                dst = bass.AP(jwap.tensor, jwap.offset + t * 160 * 128,
                              [[1024, KNN], [128, 8], [1, 16]])
                nc.sync.dma_start(
                    dst, jTi[:, :].rearrange("k (h s) -> k h s", s=16))
                src_ap = bass.AP(jwap.tensor, jwap.offset + t * 160 * 128,
                                 [[128, 160], [1, 128]])
                idq = idqp.tile([128, 160], I16, tag="idq")
                nc.sync.dma_start_transpose(idq[:, :], src_ap)
                nc.sync.dma_start(idq[16:32, :], idq[0:16, :])
                nc.sync.dma_start(idq[32:64, :], idq[0:32, :])
                nc.sync.dma_start(idq[64:128, :], idq[0:64, :])
                state[t]['idq'] = idq

            def emit_gathers(t):
                # alternate queue PAIRS per tile: consecutive tiles use
                # disjoint rings, so their DMA streams overlap instead of
                # serializing on ring-space reclaim
                g = gath.tile([128, KNN, O], gdt, tag="g")
                idq = state[t]['idq']
                qbase = 2 * (t % 2)
                for qi in range(2):
                    nc.gpsimd.dma_gather(
                        out_ap=g[:, 10 * qi:10 * qi + 10, :], in_ap=a_d[:, :],
                        idxs_ap=idq[:, 80 * qi:80 * (qi + 1)],
                        num_idxs=10 * 128, num_idxs_reg=10 * 128, elem_size=O,
                        single_packet=False, queue_num=qbase + qi)
                state[t]['g'] = g

            def emit_reduce(t):
                # contiguous max tree over k=20: 10+10 -> 5 -> (2+2)+1 -> 1
                g = state[t]['g']
                r = gath.tile([128, 10, O], gdt, tag="r", bufs=2)
                nc.vector.tensor_tensor(
                    out=r[:, :, :], in0=g[:, 0:10, :], in1=g[:, 10:20, :],
                    op=ALU.max)
                nc.vector.tensor_tensor(
                    out=r[:, 0:5, :], in0=r[:, 0:5, :], in1=r[:, 5:10, :],
                    op=ALU.max)
                nc.vector.tensor_tensor(
                    out=r[:, 0:2, :], in0=r[:, 0:2, :], in1=r[:, 2:4, :],
                    op=ALU.max)
                nc.vector.tensor_tensor(
                    out=r[:, 0, :], in0=r[:, 0, :], in1=r[:, 1, :], op=ALU.max)
                nc.vector.tensor_tensor(
                    out=m[:, t, :], in0=r[:, 0, :], in1=r[:, 4, :], op=ALU.max)
                for pi, (dst_t, orow, oc0) in enumerate(out_parts):
                    nc.tensor.matmul(
                        pxs[pi][:, t * 128:(t + 1) * 128],
                        lhsT=m[:, t, oc0:oc0 + orow], rhs=ident[:, 0:128],
                        is_transpose=True, start=False, stop=(t % 4 == 3),
                        skip_group_check=True)
                state[t] = None  # release refs

            def emit_out_half(h):
                # Prelu with per-partition bias folds the bias matmul; the px
                # bank is finalized by its last transpose (stop at t%4==3)
                cols = slice(h * 512, (h + 1) * 512)
                for pi, (dst_t, orow, oc0) in enumerate(out_parts):
                    nc.scalar.activation(
                        _r(dst_t[0:orow, cols]), pxs[pi][:, cols], AF.Prelu,
                        bias=bs[0:orow, pi:pi + 1], scale=1.0, alpha=NEG_SLOPE)

            # ---- pipelined tile loop ----
            for t in range(NT):
                emit_negSc(t)
                emit_kp(t)
                if t == 0:
                    emit_a_rows()
                if t == 1:
                    emit_c_parts()
                if t == 2 and post_tiles is not None:
                    post_tiles()
                if t >= 1:
                    emit_idq(t - 1)
                if t >= 2:
                    emit_gathers(t - 2)
                emit_topk(t)
                if t >= 3:
                    emit_reduce(t - 3)
            emit_idq(NT - 1)
            emit_gathers(NT - 2)
            emit_gathers(NT - 1)
            for t in range(NT - 3, NT):
                emit_reduce(t)
                if t == NT - 3:
                    emit_out_half(0)
                    if post_half is not None:
                        post_half(0)
            emit_out_half(1)
            if post_half is not None:
                post_half(1)

        # ---- conv5 weights staged early ----
        w5sb = {}
        for ci, (rows, k0) in enumerate([(64, 0), (64, 64), (128, 128),
                                         (128, 256), (128, 384)]):
            w5c = consts.tile([rows, 512], F32, tag=f"w5c{ci}")
            nc.sync.dma_start(_r(w5c[:, :]), _r(w5t_d[k0:k0 + rows, :]))
            w5sb[ci] = w5c
        b5sb = consts.tile([1, 512], F32, tag="b5sb")
        nc.sync.dma_start(b5sb[:, :], b5_d[:, :])
        zpart = persist.tile([128, NT, 512], F32, tag="zpart")
        zp12 = persist.tile([128, NT, 512], F32, tag="zp12")

        def zp12_fill():
            for t in range(NT):
                tcols = slice(t * 128, (t + 1) * 128)
                zp_ps = pss.tile([128, 512], F32, tag="a")
                for ci, (xt, rows) in enumerate([(x1T, 64), (x2T, 64)]):
                    nc.tensor.matmul(
                        zp_ps[:, :], lhsT=_r(xt[0:rows, tcols]),
                        rhs=_r(w5sb[ci][:, :]),
                        start=(ci == 0), stop=(ci == 1), skip_group_check=True)
                nc.scalar.activation(_r(zp12[:, t, :]), zp_ps[:, :], AF.Copy)

        def zpart_fill():
            for t in range(NT):
                tcols = slice(t * 128, (t + 1) * 128)
                zp_ps = pss.tile([128, 512], F32, tag="a")
                nc.tensor.matmul(
                    zp_ps[:, :], lhsT=_r(x3T[:, tcols]), rhs=_r(w5sb[2][:, :]),
                    start=True, stop=False, skip_group_check=True)
                nc.tensor.matmul(
                    zp_ps[:, :], lhsT=ident[:, 0:128], rhs=zp12[:, t, :],
                    start=False, stop=True, skip_group_check=True)
                nc.scalar.activation(zpart[:, t, :], zp_ps[:, :], AF.Copy)

        edge_layer(1, x0T, 3, 64, [(x1T, 64, 0)])
        edge_layer(2, x1T, 64, 64, [(x2T, 64, 0)])
        edge_layer(3, x2T, 64, 128, [(x3T, 128, 0)], post_tiles=zp12_fill)
        edge_layer(4, x3T, 128, 256, [(x4Ta, 128, 0), (x4Tb, 128, 128)],
                   post_tiles=zpart_fill)

        # ---- head: conv5 (x4 chunks; x1-x3 partials precomputed) + max pool ----
        zmax = persist.tile([128, 512], F32, tag="zmax")
        for t in range(NT):
            tcols = slice(t * 128, (t + 1) * 128)
            z_ps = pss.tile([128, 512], F32, tag="a")
            for ci, (xt, k0) in enumerate([(x4Ta, 256), (x4Tb, 384)]):
                nc.tensor.matmul(
                    z_ps[:, :], lhsT=_r(xt[:, tcols]), rhs=_r(w5sb[3 + ci][:, :]),
                    start=(ci == 0), stop=False, skip_group_check=True)
            nc.tensor.matmul(
                z_ps[:, :], lhsT=onesrow[0:1, tcols],
                rhs=b5sb[:, :], start=False, stop=True, skip_group_check=True)
            zsb = sb.tile([128, 512], F32, tag="zsb")
            nc.vector.tensor_tensor(
                out=zsb[:, :], in0=zpart[:, t, :], in1=z_ps[:, :], op=ALU.add)
            if t == 0:
                nc.scalar.activation(zmax[:, :], zsb[:, :], AF.Copy)
            else:
                nc.vector.tensor_tensor(
                    out=zmax[:, :], in0=zmax[:, :], in1=zsb[:, :], op=ALU.max)
        # transpose zmax chunks and reduce along free dim -> yT [128, 4]
        yT = persist.tile([128, 4], F32, tag="yT")
        for cchunk in range(4):
            zt_ps = pss.tile([128, 128], F32, tag="a")
            nc.tensor.matmul(
                zt_ps[:, :], lhsT=zmax[:, cchunk * 128:(cchunk + 1) * 128],
                rhs=ident[:, 0:128], is_transpose=True, start=True, stop=True,
                skip_group_check=True)
            nc.vector.tensor_reduce(
                out=yT[:, cchunk:cchunk + 1], in_=zt_ps[:, :],
                axis=AX.X, op=ALU.max)
        yTr = persist.tile([128, 4], F32, tag="yTr")
        nc.scalar.activation(yTr[:, :], yT[:, :], AF.Prelu, alpha=NEG_SLOPE)

        # ---- FC head ----
        wfc1sb = consts.tile([128, 4, 256], F32, tag="wfc1sb")
        for c in range(4):
            nc.sync.dma_start(wfc1sb[:, c, :], wfc1_d[c * 128:(c + 1) * 128, :])
        bfc1sb = consts.tile([128, 2], F32, tag="bfc1sb")
        nc.sync.dma_start(bfc1sb[:, :], bfc1_d[:, :])
        wfc2sb = consts.tile([128, 2, 128], F32, tag="wfc2sb")
        for c in range(2):
            nc.sync.dma_start(wfc2sb[:, c, :], wfc2_d[c * 128:(c + 1) * 128, :])
        bfc2sb = consts.tile([128, 1], F32, tag="bfc2sb")
        nc.sync.dma_start(bfc2sb[:, :], bfc2_d[:, :])
        wfc3sb = consts.tile([128, 40], F32, tag="wfc3sb")
        nc.sync.dma_start(wfc3sb[:, :], wfc3_d[:, :])
        bfc3sb = consts.tile([1, 40], F32, tag="bfc3sb")
        nc.sync.dma_start(bfc3sb[:, :], bfc3_d[:, :])

        h1sb = persist.tile([128, 2], F32, tag="h1sb")
        for mt in range(2):
            h1_ps = pss.tile([128, 1], F32, tag="a")
            for c in range(4):
                nc.tensor.matmul(
                    h1_ps[:, :], lhsT=wfc1sb[:, c, mt * 128:(mt + 1) * 128],
                    rhs=yTr[:, c:c + 1],
                    start=(c == 0), stop=(c == 3), skip_group_check=True)
            nc.scalar.activation(
                h1sb[:, mt:mt + 1], h1_ps[:, :], AF.Prelu,
                bias=bfc1sb[:, mt:mt + 1], scale=1.0, alpha=NEG_SLOPE)
        h2sb = persist.tile([128, 1], F32, tag="h2sb")
        h2_ps = pss.tile([128, 1], F32, tag="a")
        for c in range(2):
            nc.tensor.matmul(
                h2_ps[:, :], lhsT=wfc2sb[:, c, :], rhs=h1sb[:, c:c + 1],
                start=(c == 0), stop=(c == 1), skip_group_check=True)
        nc.scalar.activation(
            h2sb[:, :], h2_ps[:, :], AF.Prelu,
            bias=bfc2sb[:, :], scale=1.0, alpha=NEG_SLOPE)

        out_ps = pss.tile([40, 1], F32, tag="a")
        nc.tensor.matmul(
            out_ps[:, :], lhsT=wfc3sb[:, :], rhs=h2sb[:, :],
            start=True, stop=False, skip_group_check=True)
        nc.tensor.matmul(
            out_ps[:, :], lhsT=bfc3sb[:, :], rhs=onescol[0:1, :],
            start=False, stop=True, skip_group_check=True)
        out_sb = persist.tile([40, 1], F32, tag="out_sb")
        nc.scalar.activation(out_sb[:, :], out_ps[:, :], AF.Copy)
        nc.sync.dma_start(out_d[:, :], out_sb[:, :])


# ---------------------------------------------------------------------------
# harness entry point
# ---------------------------------------------------------------------------
_NC_CACHE = {}


def _get_nc():
    if 'nc' not in _NC_CACHE:
        _NC_CACHE['nc'] = build_nc()
    return _NC_CACHE['nc']


def kernel(**inputs):
    """Full-batch EdgeCNN forward. x: (8, 1024, 3) -> (8, 40) float32.

    Pure data parallel: batch element b runs on NeuronCore b.
    """
    from concourse.bass_utils import run_bass_kernel_spmd

    inp = {k: np.asarray(v) for k, v in inputs.items()}
    prep = host_prep(inp)
    nc = _get_nc()
    in_maps = []
    for b in range(8):
        m = {'x': np.ascontiguousarray(inp['x'][b]).astype(np.float32)}
        m.update(prep)
        in_maps.append(m)
    res = run_bass_kernel_spmd(nc, in_maps, core_ids=list(range(8)))
    out = np.stack([res.results[b]['out'].reshape(40) for b in range(8)])
    return out.astype(np.float32)


# revision 25
# speedup vs baseline: 1.0916x; 1.0916x over previous
"""EdgeCNN (DGCNN) Bass/Tile kernel for TRN2 — one batch element per core.

Per edge-conv layer (N=1024 points, K=20 neighbors):
  1. PE: augmented matmul key[n,j] = 2<xn,xj> - S[j]  (ones-row trick folds
     the -S[j] column term; the -S[n] row term is rank-invariant and dropped)
  2. ACT: evict keys PSUM -> SBUF
  3. GPSIMD: AND low-10 mantissa bits, OR in column index j -> packed keys
  4. DVE: 3x max8 + 2x match_replace -> top-20 packed keys; extract j
  5. idx -> DRAM -> read back wrapped (partition = i%16); dma_gather of
     a = x @ (g~ Wn)^T rows SPLIT 4-WAY across SWDGE queues 0-3 (concurrent
     Q7 cpu pairs + DMA paths)
  6. DVE: strided reduce_max over k; PE: transpose(m) + c-matmul accumulate
  7. ACT: leaky-relu (Prelu alpha=0.2) PSUM -> next layer xT
Head: conv5 via K-chunk accumulation, global max-pool, 3 FC layers on PE.
"""

import contextlib

import numpy as np

import concourse.bass as bass
import concourse.bacc as bacc
import concourse.mybir as mybir
from concourse.tile import TileContext
from concourse.masks import make_identity

F32 = mybir.dt.float32
U32 = mybir.dt.uint32
I16 = mybir.dt.int16
F16 = mybir.dt.float16
F32R = mybir.dt.float32r


def _r(ap):
    return ap.bitcast(F32R)
AF = mybir.ActivationFunctionType
ALU = mybir.AluOpType
AX = mybir.AxisListType

N = 1024
KNN = 20
NT = 8
NQ = 4            # SWDGE queues
NEG_SLOPE = 0.2
BNI = np.float32(1.0 / np.sqrt(1.0 + 1e-5))
LAYERS = [(3, 64), (64, 64), (64, 128), (128, 256)]
NEG_BIG = -3.0e38


def host_prep(inp):
    """Fold BN scale/bias into weights; transpose for device layout."""
    d = {}
    for li, (C, O) in enumerate(LAYERS, start=1):
        W = inp[f'W{li}'].astype(np.float32)
        g = inp[f'g{li}'].astype(np.float32)
        b = inp[f'b{li}'].astype(np.float32)
        gt = g * BNI
        Wn = W[:, :C]
        Wc = W[:, C:]
        d[f'wnt{li}'] = np.ascontiguousarray((gt[:, None] * Wn).T)          # (C, O)
        d[f'wdt{li}'] = np.ascontiguousarray((gt[:, None] * (Wc - Wn)).T)   # (C, O)
        d[f'bs{li}'] = np.ascontiguousarray(b.reshape(max(1, O // 128), min(O, 128)).T)
    g5 = inp['g5'].astype(np.float32) * BNI
    d['w5t'] = np.ascontiguousarray((g5[:, None] * inp['W5']).T)            # (512, 512)
    d['b5'] = inp['b5'].reshape(1, 512).astype(np.float32).copy()
    g1 = inp['bng1'].astype(np.float32) * BNI
    d['wfc1'] = np.ascontiguousarray((g1[:, None] * inp['fc1_w']).T)        # (512, 256)
    bf1 = g1 * inp['fc1_b'].astype(np.float32) + inp['bnb1'].astype(np.float32)
    d['bfc1'] = np.ascontiguousarray(bf1.reshape(2, 128).T)                 # (128, 2)
    g2 = inp['bng2'].astype(np.float32) * BNI
    d['wfc2'] = np.ascontiguousarray((g2[:, None] * inp['fc2_w']).T)        # (256, 128)
    bf2 = g2 * inp['fc2_b'].astype(np.float32) + inp['bnb2'].astype(np.float32)
    d['bfc2'] = np.ascontiguousarray(bf2.reshape(128, 1))                   # (128, 1)
    d['wfc3'] = np.ascontiguousarray(inp['fc3_w'].T)                        # (128, 40)
    d['bfc3'] = inp['fc3_b'].reshape(1, 40).astype(np.float32).copy()
    return d


def build_nc():
    nc = bacc.Bacc("TRN2", target_bir_lowering=False, debug=False, num_devices=8,
                   num_swdge_queues=NQ)
    with TileContext(nc) as tc:
        _trace(nc, tc)
    nc.compile()
    return nc


def _trace(nc, tc):
    with contextlib.ExitStack() as ctx:
        dram = ctx.enter_context(tc.tile_pool(name="dram", bufs=1, space="DRAM"))
        consts = ctx.enter_context(tc.tile_pool(name="consts", bufs=1))
        persist = ctx.enter_context(tc.tile_pool(name="persist", bufs=1))
        sb = ctx.enter_context(tc.tile_pool(name="sb", bufs=2))
        kpcp = ctx.enter_context(tc.tile_pool(name="kpcp", bufs=2))
        keyp = ctx.enter_context(tc.tile_pool(name="keyp", bufs=2))
        smalls = ctx.enter_context(tc.tile_pool(name="smalls", bufs=4))
        idqp = ctx.enter_context(tc.tile_pool(name="idqp", bufs=4))
        gath = ctx.enter_context(tc.tile_pool(name="gath", bufs=3))
        mp = ctx.enter_context(tc.tile_pool(name="mp", bufs=1))
        psb = ctx.enter_context(tc.tile_pool(name="psb", bufs=1, space="PSUM"))
        pss = ctx.enter_context(tc.tile_pool(name="pss", bufs=2, space="PSUM"))

        # ---- DRAM I/O ----
        x_d = dram.tile([N, 3], F32, kind="ExternalInput", uniquify=False, name="x")
        win = {}
        for li, (C, O) in enumerate(LAYERS, start=1):
            win[f'wnt{li}'] = dram.tile([C, O], F32, kind="ExternalInput", uniquify=False, name=f"wnt{li}")
            win[f'wdt{li}'] = dram.tile([C, O], F32, kind="ExternalInput", uniquify=False, name=f"wdt{li}")
            win[f'bs{li}'] = dram.tile([min(O, 128), max(1, O // 128)], F32, kind="ExternalInput", uniquify=False, name=f"bs{li}")
        w5t_d = dram.tile([512, 512], F32, kind="ExternalInput", uniquify=False, name="w5t")
        b5_d = dram.tile([1, 512], F32, kind="ExternalInput", uniquify=False, name="b5")
        wfc1_d = dram.tile([512, 256], F32, kind="ExternalInput", uniquify=False, name="wfc1")
        bfc1_d = dram.tile([128, 2], F32, kind="ExternalInput", uniquify=False, name="bfc1")
        wfc2_d = dram.tile([256, 128], F32, kind="ExternalInput", uniquify=False, name="wfc2")
        bfc2_d = dram.tile([128, 1], F32, kind="ExternalInput", uniquify=False, name="bfc2")
        wfc3_d = dram.tile([128, 40], F32, kind="ExternalInput", uniquify=False, name="wfc3")
        bfc3_d = dram.tile([1, 40], F32, kind="ExternalInput", uniquify=False, name="bfc3")
        out_d = dram.tile([40, 1], F32, kind="ExternalOutput", uniquify=False, name="out")

        gdts = {1: F32, 2: F32, 3: F16, 4: F16}
        a_ds = {li: dram.tile([N, O], gdts[li], name=f"a_d{li}")
                for li, (C, O) in enumerate(LAYERS, start=1)}
        jw_ds = {li: dram.tile([N * KNN // 16, 128], I16, name=f"jw_d{li}")
                 for li in range(1, 5)}

        # ---- consts ----
        iotaJ = consts.tile([128, N], U32, tag="iotaJ")
        nc.gpsimd.iota(iotaJ[:, :], [[1, N]], base=0, channel_multiplier=0)
        ident = consts.tile([128, 128], F32, tag="ident")
        make_identity(nc, ident[:, :])
        onescol = consts.tile([128, 1], F32, tag="onescol")
        nc.vector.memset(onescol[:, :], 1.0)
        onescolR = consts.tile([128, 1], F32, tag="onescolR")
        nc.sync.dma_start(_r(onescolR[:, :]), _r(onescol[:, :]))
        onesrow = consts.tile([1, N], F32, tag="onesrow")
        nc.vector.memset(onesrow[:, :], 1.0)
        onesrowR = consts.tile([1, N], F32, tag="onesrowR")
        nc.sync.dma_start(_r(onesrowR[:, :]), _r(onesrow[:, :]))

        # persistent feature tensors (augmented with a trailing ones row
        # where the next layer uses the ones-trick, i.e. C_next + 1 <= 128)
        x0T = persist.tile([4, N], F32, tag="x0T")
        x1T = persist.tile([65, N], F32, tag="x1T")
        x2T = persist.tile([65, N], F32, tag="x2T")
        x3T = persist.tile([128, N], F32, tag="x3T")
        x4Ta = persist.tile([128, N], F32, tag="x4Ta")
        x4Tb = persist.tile([128, N], F32, tag="x4Tb")
        nc.sync.dma_start(_r(x0T[3:4, :]), _r(onesrow[0:1, :]))
        nc.sync.dma_start(_r(x1T[64:65, :]), _r(onesrow[0:1, :]))
        nc.sync.dma_start(_r(x2T[64:65, :]), _r(onesrow[0:1, :]))

        # load x transposed: x_d is (N, 3) row-major
        xap = x_d[:, :]
        nc.sync.dma_start(
            _r(x0T[0:3, :]),
            bass.AP(xap.tensor, xap.offset, [[1, 3], [3, N]]).bitcast(F32R))

        def edge_layer(li, xT, C, O, out_parts, post_tiles=None,
                       post_half=None):
            """xT: [C(+1), N] features (row C = ones iff aug). out_parts:
            list of (dest_ap, orow, ocol0)."""
            aug = (C + 1 <= 128) and li < 4
            gdt = gdts[li]
            a_d = a_ds[li]
            jw_d = jw_ds[li]
            jwap = jw_d[:, :]
            wnt = sb.tile([C, O], F32, tag="wnt")
            wdt = sb.tile([C, O], F32, tag="wdt")
            bs = sb.tile([min(O, 128), max(1, O // 128)], F32, tag="bs")
            nc.sync.dma_start(_r(wnt[:, :]), _r(win[f'wnt{li}'][:, :]))
            nc.sync.dma_start(_r(wdt[:, :]), _r(win[f'wdt{li}'][:, :]))
            nc.sync.dma_start(bs[:, :], win[f'bs{li}'][:, :])

            xsq = sb.tile([C, N], F32, tag="xsq")
            if aug:
                x2dA = sb.tile([C + 1, N], F32, tag="x2dA")
            else:
                x2dA = sb.tile([C, N], F32, tag="x2dA")
            # compute engines may only start at partition 0/32/64/96: stage
            # negS in a [1, N] tile and DMA into the augmented row otherwise
            direct = aug and C % 32 == 0
            negS = None if direct else sb.tile([1, N], F32, tag="negS")
            for h in range(2):
                cols = slice(h * 512, (h + 1) * 512)
                nc.scalar.activation(_r(xsq[:, cols]), xT[0:C, cols], AF.Square)
                nc.scalar.activation(
                    _r(x2dA[0:C, cols]), xT[0:C, cols], AF.Copy, bias=0.0,
                    scale=2.0)
                S_ps = pss.tile([1, 512], F32, tag="a")
                nc.tensor.matmul(
                    S_ps[:, :], lhsT=_r(onescolR[0:C, :]), rhs=_r(xsq[:, cols]),
                    start=True, stop=True, skip_group_check=True)
                dst = x2dA[C:C + 1, cols] if direct else negS[0:1, cols]
                nc.scalar.activation(_r(dst), S_ps[:, :], AF.Copy, bias=0.0,
                                     scale=-1.0)
                if aug and not direct:
                    nc.sync.dma_start(_r(x2dA[C:C + 1, cols]), _r(negS[0:1, cols]))

            negSc = sb.tile([128, NT], F32, tag="negSc")

            def emit_negSc(t):
                nsp = pss.tile([128, 1], F32, tag="a")
                nc.tensor.matmul(
                    nsp[:, :], lhsT=xsq[:, t * 128:(t + 1) * 128],
                    rhs=onescol[0:C, :],
                    start=True, stop=True, skip_group_check=True)
                nc.scalar.activation(
                    negSc[:, t:t + 1], nsp[:, :], AF.Copy, bias=0.0, scale=-1.0)

            # px PSUM tiles; c-part matmuls issued early (start=True)
            pxs = []
            for (dst_ap, orow, oc0) in out_parts:
                px = psb.tile([orow, N], F32, tag=f"px{oc0}")
                pxs.append(px)

            m = mp.tile([128, NT, O], F32, tag=f"m{li}")

            state = {}

            def emit_a_rows():
                # block a-tiles into [128, 512]-col PSUM groups: one eviction
                # and one (3-level AP) DMA per group instead of per tile
                tpg = 512 // O                      # tiles per group
                for g0 in range(0, NT, tpg):
                    ng = min(tpg, NT - g0)
                    a_ps = pss.tile([128, ng, O], F32, tag="a")
                    for ti in range(ng):
                        lt = xT[0:C, (g0 + ti) * 128:(g0 + ti + 1) * 128]
                        wv = wnt[:, :]
                        if O >= 256:
                            lt, wv = _r(lt), _r(wv)
                        nc.tensor.matmul(
                            a_ps[:, ti, :], lhsT=lt, rhs=wv,
                            start=True, stop=True, skip_group_check=True)
                    a_sb = sb.tile([128, ng, O], gdt, tag="a_sb")
                    nc.scalar.activation(
                        a_sb[:, :, :].rearrange("p t o -> p (t o)"),
                        a_ps[:, :, :].rearrange("p t o -> p (t o)"), AF.Copy)
                    adap = a_d[:, :]
                    dst = bass.AP(adap.tensor, adap.offset + g0 * 128 * O,
                                  [[O, 128], [128 * O, ng], [1, O]])
                    nc.sync.dma_start(dst, a_sb[:, :, :])

            def emit_c_parts():
                for pi, (dst_t, orow, oc0) in enumerate(out_parts):
                    for h in range(2):
                        cols = slice(h * 512, (h + 1) * 512)
                        nc.tensor.matmul(
                            pxs[pi][:, cols],
                            lhsT=_r(wdt[:, oc0:oc0 + orow]), rhs=_r(xT[0:C, cols]),
                            start=True, stop=False, skip_group_check=True)

            def emit_kp(t):
                tcols = slice(t * 128, (t + 1) * 128)
                kp = psb.tile([128, N], F32, tag="kp")
                for h in range(2):
                    cols = slice(h * 512, (h + 1) * 512)
                    if aug:
                        nc.tensor.matmul(
                            kp[:, cols], lhsT=_r(xT[:, tcols]), rhs=_r(x2dA[:, cols]),
                            start=True, stop=True, skip_group_check=True)
                    else:
                        nc.tensor.matmul(
                            kp[:, cols], lhsT=_r(xT[0:C, tcols]), rhs=_r(x2dA[:, cols]),
                            start=True, stop=False, skip_group_check=True)
                        nc.tensor.matmul(
                            kp[:, cols], lhsT=_r(onesrowR[0:1, tcols]),
                            rhs=_r(negS[0:1, cols]),
                            start=False, stop=True, skip_group_check=True)
                kpc = kpcp.tile([128, N], F32, tag="kpc")
                nc.scalar.activation(
                    kpc[:, :], kp[:, :], AF.Prelu, bias=negSc[:, t:t + 1],
                    scale=1.0, alpha=1.0)
                kb = keyp.tile([128, N], U32, tag="kb")
                nc.vector.tensor_scalar(
                    kb[:, :], kpc[:, :].bitcast(U32), 0xFFFFFC00, None,
                    op0=ALU.bitwise_and)
                nc.vector.tensor_tensor(
                    out=kb[:, :], in0=kb[:, :], in1=iotaJ[:, :], op=ALU.bitwise_or)
                state[t] = {'kb': kb}

            def emit_topk(t):
                kbf = state[t]['kb'][:, :].bitcast(F32)
                v24 = smalls.tile([128, 24], F32, tag="v24")
                nc.vector.max(v24[:, 0:8], kbf)
                nc.vector.match_replace(kbf, v24[:, 0:8], kbf, NEG_BIG)
                nc.vector.max(v24[:, 8:16], kbf)
                nc.vector.match_replace(kbf, v24[:, 8:16], kbf, NEG_BIG)
                nc.vector.max(v24[:, 16:24], kbf)
                j20 = smalls.tile([128, KNN], U32, tag="j20")
                nc.vector.tensor_scalar(
                    j20[:, :], v24[:, 0:KNN].bitcast(U32), 0x3FF, None,
                    op0=ALU.bitwise_and)
                jf20 = smalls.tile([128, KNN], F32, tag="jf20")
                nc.vector.tensor_copy(jf20[:, :], j20[:, :])
                state[t]['jf20'] = jf20

            def emit_idq(t):
                jT_ps = pss.tile([KNN, 128], F32, tag="a")
                nc.tensor.matmul(
                    jT_ps[:, :], lhsT=state[t]['jf20'][:, :], rhs=ident[:, 0:128],
                    is_transpose=True, start=True, stop=True, skip_group_check=True)
                jTi = smalls.tile([KNN, 128], I16, tag="jTi")
                nc.scalar.activation(jTi[:, :], jT_ps[:, :], AF.Copy)
                dst = bass.AP(jwap.tensor, jwap.offset + t * 160 * 128,
                              [[1024, KNN], [128, 8], [1, 16]])
                nc.sync.dma_start(
                    dst, jTi[:, :].rearrange("k (h s) -> k h s", s=16))
                src_ap = bass.AP(jwap.tensor, jwap.offset + t * 160 * 128,
                                 [[128, 160], [1, 128]])
                idq = idqp.tile([128, 160], I16, tag="idq")
                nc.sync.dma_start_transpose(idq[:, :], src_ap)
                nc.sync.dma_start(idq[16:32, :], idq[0:16, :])
                nc.sync.dma_start(idq[32:64, :], idq[0:32, :])
                nc.sync.dma_start(idq[64:128, :], idq[0:64, :])
                state[t]['idq'] = idq

            def emit_gathers(t):
                # 10 sub-gathers of 256 descs round-robin over the 4 queues:
                # ring entries stay small so reclaim never blocks, and
                # consecutive tiles' DMA streams pipeline on all queues
                g = gath.tile([128, KNN, O], gdt, tag="g")
                idq = state[t]['idq']
                for s in range(10):
                    nc.gpsimd.dma_gather(
                        out_ap=g[:, 2 * s:2 * s + 2, :], in_ap=a_d[:, :],
                        idxs_ap=idq[:, 16 * s:16 * (s + 1)],
                        num_idxs=256, num_idxs_reg=256, elem_size=O,
                        single_packet=False, queue_num=s % NQ)
                state[t]['g'] = g

            def emit_reduce(t):
                # contiguous max tree over k=20: 10+10 -> 5 -> (2+2)+1 -> 1
                g = state[t]['g']
                r = gath.tile([128, 10, O], gdt, tag="r", bufs=2)
                nc.vector.tensor_tensor(
                    out=r[:, :, :], in0=g[:, 0:10, :], in1=g[:, 10:20, :],
                    op=ALU.max)
                nc.vector.tensor_tensor(
                    out=r[:, 0:5, :], in0=r[:, 0:5, :], in1=r[:, 5:10, :],
                    op=ALU.max)
                nc.vector.tensor_tensor(
                    out=r[:, 0:2, :], in0=r[:, 0:2, :], in1=r[:, 2:4, :],
                    op=ALU.max)
                nc.vector.tensor_tensor(
                    out=r[:, 0, :], in0=r[:, 0, :], in1=r[:, 1, :], op=ALU.max)
                nc.vector.tensor_tensor(
                    out=m[:, t, :], in0=r[:, 0, :], in1=r[:, 4, :], op=ALU.max)
                for pi, (dst_t, orow, oc0) in enumerate(out_parts):
                    nc.tensor.matmul(
                        pxs[pi][:, t * 128:(t + 1) * 128],
                        lhsT=m[:, t, oc0:oc0 + orow], rhs=ident[:, 0:128],
                        is_transpose=True, start=False, stop=(t % 4 == 3),
                        skip_group_check=True)
                state[t] = None  # release refs

            def emit_out_half(h):
                # Prelu with per-partition bias folds the bias matmul; the px
                # bank is finalized by its last transpose (stop at t%4==3)
                cols = slice(h * 512, (h + 1) * 512)
                for pi, (dst_t, orow, oc0) in enumerate(out_parts):
                    nc.scalar.activation(
                        _r(dst_t[0:orow, cols]), pxs[pi][:, cols], AF.Prelu,
                        bias=bs[0:orow, pi:pi + 1], scale=1.0, alpha=NEG_SLOPE)

            # ---- pipelined tile loop ----
            for t in range(NT):
                emit_negSc(t)
                emit_kp(t)
                if t == 0:
                    emit_a_rows()
                if t == 1:
                    emit_c_parts()
                if t == 2 and post_tiles is not None:
                    post_tiles()
                if t >= 1:
                    emit_idq(t - 1)
                if t >= 2:
                    emit_gathers(t - 2)
                emit_topk(t)
                if t >= 3:
                    emit_reduce(t - 3)
            emit_idq(NT - 1)
            emit_gathers(NT - 2)
            emit_gathers(NT - 1)
            for t in range(NT - 3, NT):
                emit_reduce(t)
                if t == NT - 3:
                    emit_out_half(0)
                    if post_half is not None:
                        post_half(0)
            emit_out_half(1)
            if post_half is not None:
                post_half(1)

        # ---- conv5 weights staged early ----
        w5sb = {}
        for ci, (rows, k0) in enumerate([(64, 0), (64, 64), (128, 128),
                                         (128, 256), (128, 384)]):
            w5c = consts.tile([rows, 512], F32, tag=f"w5c{ci}")
            nc.sync.dma_start(_r(w5c[:, :]), _r(w5t_d[k0:k0 + rows, :]))
            w5sb[ci] = w5c
        b5sb = consts.tile([1, 512], F32, tag="b5sb")
        nc.sync.dma_start(b5sb[:, :], b5_d[:, :])
        zpart = persist.tile([128, NT, 512], F32, tag="zpart")
        zp12 = persist.tile([128, NT, 512], F32, tag="zp12")

        def zp12_fill():
            for t in range(NT):
                tcols = slice(t * 128, (t + 1) * 128)
                zp_ps = pss.tile([128, 512], F32, tag="a")
                for ci, (xt, rows) in enumerate([(x1T, 64), (x2T, 64)]):
                    nc.tensor.matmul(
                        zp_ps[:, :], lhsT=_r(xt[0:rows, tcols]),
                        rhs=_r(w5sb[ci][:, :]),
                        start=(ci == 0), stop=(ci == 1), skip_group_check=True)
                nc.scalar.activation(_r(zp12[:, t, :]), zp_ps[:, :], AF.Copy)

        def zpart_fill():
            for t in range(NT):
                tcols = slice(t * 128, (t + 1) * 128)
                zp_ps = pss.tile([128, 512], F32, tag="a")
                nc.tensor.matmul(
                    zp_ps[:, :], lhsT=_r(x3T[:, tcols]), rhs=_r(w5sb[2][:, :]),
                    start=True, stop=False, skip_group_check=True)
                nc.tensor.matmul(
                    zp_ps[:, :], lhsT=ident[:, 0:128], rhs=zp12[:, t, :],
                    start=False, stop=True, skip_group_check=True)
                nc.scalar.activation(zpart[:, t, :], zp_ps[:, :], AF.Copy)

        edge_layer(1, x0T, 3, 64, [(x1T, 64, 0)])
        edge_layer(2, x1T, 64, 64, [(x2T, 64, 0)])
        edge_layer(3, x2T, 64, 128, [(x3T, 128, 0)], post_tiles=zp12_fill)
        edge_layer(4, x3T, 128, 256, [(x4Ta, 128, 0), (x4Tb, 128, 128)],
                   post_tiles=zpart_fill)

        # ---- head: conv5 (x4 chunks; x1-x3 partials precomputed) + max pool ----
        zmax = persist.tile([128, 512], F32, tag="zmax")
        for t in range(NT):
            tcols = slice(t * 128, (t + 1) * 128)
            z_ps = pss.tile([128, 512], F32, tag="a")
            for ci, (xt, k0) in enumerate([(x4Ta, 256), (x4Tb, 384)]):
                nc.tensor.matmul(
                    z_ps[:, :], lhsT=_r(xt[:, tcols]), rhs=_r(w5sb[3 + ci][:, :]),
                    start=(ci == 0), stop=False, skip_group_check=True)
            nc.tensor.matmul(
                z_ps[:, :], lhsT=onesrow[0:1, tcols],
                rhs=b5sb[:, :], start=False, stop=True, skip_group_check=True)
            zsb = sb.tile([128, 512], F32, tag="zsb")
            nc.vector.tensor_tensor(
                out=zsb[:, :], in0=zpart[:, t, :], in1=z_ps[:, :], op=ALU.add)
            if t == 0:
                nc.scalar.activation(zmax[:, :], zsb[:, :], AF.Copy)
            else:
                nc.vector.tensor_tensor(
                    out=zmax[:, :], in0=zmax[:, :], in1=zsb[:, :], op=ALU.max)
        # transpose zmax chunks and reduce along free dim -> yT [128, 4]
        yT = persist.tile([128, 4], F32, tag="yT")
        for cchunk in range(4):
            zt_ps = pss.tile([128, 128], F32, tag="a")
            nc.tensor.matmul(
                zt_ps[:, :], lhsT=zmax[:, cchunk * 128:(cchunk + 1) * 128],
                rhs=ident[:, 0:128], is_transpose=True, start=True, stop=True,
                skip_group_check=True)
            nc.vector.tensor_reduce(
                out=yT[:, cchunk:cchunk + 1], in_=zt_ps[:, :],
                axis=AX.X, op=ALU.max)
        yTr = persist.tile([128, 4], F32, tag="yTr")
        nc.scalar.activation(yTr[:, :], yT[:, :], AF.Prelu, alpha=NEG_SLOPE)

        # ---- FC head ----
        wfc1sb = consts.tile([128, 4, 256], F32, tag="wfc1sb")
        for c in range(4):
            nc.sync.dma_start(wfc1sb[:, c, :], wfc1_d[c * 128:(c + 1) * 128, :])
        bfc1sb = consts.tile([128, 2], F32, tag="bfc1sb")
        nc.sync.dma_start(bfc1sb[:, :], bfc1_d[:, :])
        wfc2sb = consts.tile([128, 2, 128], F32, tag="wfc2sb")
        for c in range(2):
            nc.sync.dma_start(wfc2sb[:, c, :], wfc2_d[c * 128:(c + 1) * 128, :])
        bfc2sb = consts.tile([128, 1], F32, tag="bfc2sb")
        nc.sync.dma_start(bfc2sb[:, :], bfc2_d[:, :])
        wfc3sb = consts.tile([128, 40], F32, tag="wfc3sb")
        nc.sync.dma_start(wfc3sb[:, :], wfc3_d[:, :])
        bfc3sb = consts.tile([1, 40], F32, tag="bfc3sb")
        nc.sync.dma_start(bfc3sb[:, :], bfc3_d[:, :])

        h1sb = persist.tile([128, 2], F32, tag="h1sb")
        for mt in range(2):
            h1_ps = pss.tile([128, 1], F32, tag="a")
            for c in range(4):
                nc.tensor.matmul(
                    h1_ps[:, :], lhsT=wfc1sb[:, c, mt * 128:(mt + 1) * 128],
                    rhs=yTr[:, c:c + 1],
                    start=(c == 0), stop=(c == 3), skip_group_check=True)
            nc.scalar.activation(
                h1sb[:, mt:mt + 1], h1_ps[:, :], AF.Prelu,
                bias=bfc1sb[:, mt:mt + 1], scale=1.0, alpha=NEG_SLOPE)
        h2sb = persist.tile([128, 1], F32, tag="h2sb")
        h2_ps = pss.tile([128, 1], F32, tag="a")
        for c in range(2):
            nc.tensor.matmul(
                h2_ps[:, :], lhsT=wfc2sb[:, c, :], rhs=h1sb[:, c:c + 1],
                start=(c == 0), stop=(c == 1), skip_group_check=True)
        nc.scalar.activation(
            h2sb[:, :], h2_ps[:, :], AF.Prelu,
            bias=bfc2sb[:, :], scale=1.0, alpha=NEG_SLOPE)

        out_ps = pss.tile([40, 1], F32, tag="a")
        nc.tensor.matmul(
            out_ps[:, :], lhsT=wfc3sb[:, :], rhs=h2sb[:, :],
            start=True, stop=False, skip_group_check=True)
        nc.tensor.matmul(
            out_ps[:, :], lhsT=bfc3sb[:, :], rhs=onescol[0:1, :],
            start=False, stop=True, skip_group_check=True)
        out_sb = persist.tile([40, 1], F32, tag="out_sb")
        nc.scalar.activation(out_sb[:, :], out_ps[:, :], AF.Copy)
        nc.sync.dma_start(out_d[:, :], out_sb[:, :])


# ---------------------------------------------------------------------------
# harness entry point
# ---------------------------------------------------------------------------
_NC_CACHE = {}


def _get_nc():
    if 'nc' not in _NC_CACHE:
        _NC_CACHE['nc'] = build_nc()
    return _NC_CACHE['nc']


def kernel(**inputs):
    """Full-batch EdgeCNN forward. x: (8, 1024, 3) -> (8, 40) float32.

    Pure data parallel: batch element b runs on NeuronCore b.
    """
    from concourse.bass_utils import run_bass_kernel_spmd

    inp = {k: np.asarray(v) for k, v in inputs.items()}
    prep = host_prep(inp)
    nc = _get_nc()
    in_maps = []
    for b in range(8):
        m = {'x': np.ascontiguousarray(inp['x'][b]).astype(np.float32)}
        m.update(prep)
        in_maps.append(m)
    res = run_bass_kernel_spmd(nc, in_maps, core_ids=list(range(8)))
    out = np.stack([res.results[b]['out'].reshape(40) for b in range(8)])
    return out.astype(np.float32)


# revision 27
# speedup vs baseline: 1.1387x; 1.0431x over previous
"""EdgeCNN (DGCNN) Bass/Tile kernel for TRN2 — one batch element per core.

Per edge-conv layer (N=1024 points, K=20 neighbors):
  1. PE: augmented matmul key[n,j] = 2<xn,xj> - S[j]  (ones-row trick folds
     the -S[j] column term; the -S[n] row term is rank-invariant and dropped)
  2. ACT: evict keys PSUM -> SBUF
  3. GPSIMD: AND low-10 mantissa bits, OR in column index j -> packed keys
  4. DVE: 3x max8 + 2x match_replace -> top-20 packed keys; extract j
  5. idx -> DRAM -> read back wrapped (partition = i%16); dma_gather of
     a = x @ (g~ Wn)^T rows SPLIT 4-WAY across SWDGE queues 0-3 (concurrent
     Q7 cpu pairs + DMA paths)
  6. DVE: strided reduce_max over k; PE: transpose(m) + c-matmul accumulate
  7. ACT: leaky-relu (Prelu alpha=0.2) PSUM -> next layer xT
Head: conv5 via K-chunk accumulation, global max-pool, 3 FC layers on PE.
"""

import contextlib

import numpy as np

import concourse.bass as bass
import concourse.bacc as bacc
import concourse.mybir as mybir
from concourse.tile import TileContext
from concourse.masks import make_identity

F32 = mybir.dt.float32
U32 = mybir.dt.uint32
I16 = mybir.dt.int16
F16 = mybir.dt.float16
F32R = mybir.dt.float32r


def _r(ap):
    return ap.bitcast(F32R)
AF = mybir.ActivationFunctionType
ALU = mybir.AluOpType
AX = mybir.AxisListType

N = 1024
KNN = 20
NT = 8
NQ = 4            # SWDGE queues
NEG_SLOPE = 0.2
BNI = np.float32(1.0 / np.sqrt(1.0 + 1e-5))
LAYERS = [(3, 64), (64, 64), (64, 128), (128, 256)]
NEG_BIG = -3.0e38


def host_prep(inp):
    """Fold BN scale/bias into weights; transpose for device layout."""
    d = {}
    for li, (C, O) in enumerate(LAYERS, start=1):
        W = inp[f'W{li}'].astype(np.float32)
        g = inp[f'g{li}'].astype(np.float32)
        b = inp[f'b{li}'].astype(np.float32)
        gt = g * BNI
        Wn = W[:, :C]
        Wc = W[:, C:]
        d[f'wnt{li}'] = np.ascontiguousarray((gt[:, None] * Wn).T)          # (C, O)
        d[f'wdt{li}'] = np.ascontiguousarray((gt[:, None] * (Wc - Wn)).T)   # (C, O)
        d[f'bs{li}'] = np.ascontiguousarray(b.reshape(max(1, O // 128), min(O, 128)).T)
    g5 = inp['g5'].astype(np.float32) * BNI
    d['w5t'] = np.ascontiguousarray((g5[:, None] * inp['W5']).T)            # (512, 512)
    d['b5'] = inp['b5'].reshape(1, 512).astype(np.float32).copy()
    g1 = inp['bng1'].astype(np.float32) * BNI
    d['wfc1'] = np.ascontiguousarray((g1[:, None] * inp['fc1_w']).T)        # (512, 256)
    bf1 = g1 * inp['fc1_b'].astype(np.float32) + inp['bnb1'].astype(np.float32)
    d['bfc1'] = np.ascontiguousarray(bf1.reshape(2, 128).T)                 # (128, 2)
    g2 = inp['bng2'].astype(np.float32) * BNI
    d['wfc2'] = np.ascontiguousarray((g2[:, None] * inp['fc2_w']).T)        # (256, 128)
    bf2 = g2 * inp['fc2_b'].astype(np.float32) + inp['bnb2'].astype(np.float32)
    d['bfc2'] = np.ascontiguousarray(bf2.reshape(128, 1))                   # (128, 1)
    d['wfc3'] = np.ascontiguousarray(inp['fc3_w'].T)                        # (128, 40)
    d['bfc3'] = inp['fc3_b'].reshape(1, 40).astype(np.float32).copy()
    return d


def build_nc():
    nc = bacc.Bacc("TRN2", target_bir_lowering=False, debug=False, num_devices=8,
                   num_swdge_queues=NQ)
    with TileContext(nc) as tc:
        _trace(nc, tc)
    nc.compile()
    return nc


def _trace(nc, tc):
    with contextlib.ExitStack() as ctx:
        dram = ctx.enter_context(tc.tile_pool(name="dram", bufs=1, space="DRAM"))
        consts = ctx.enter_context(tc.tile_pool(name="consts", bufs=1))
        persist = ctx.enter_context(tc.tile_pool(name="persist", bufs=1))
        sb = ctx.enter_context(tc.tile_pool(name="sb", bufs=2))
        kpcp = ctx.enter_context(tc.tile_pool(name="kpcp", bufs=2))
        keyp = ctx.enter_context(tc.tile_pool(name="keyp", bufs=2))
        smalls = ctx.enter_context(tc.tile_pool(name="smalls", bufs=4))
        idqp = ctx.enter_context(tc.tile_pool(name="idqp", bufs=4))
        gath = ctx.enter_context(tc.tile_pool(name="gath", bufs=3))
        mp = ctx.enter_context(tc.tile_pool(name="mp", bufs=1))
        psb = ctx.enter_context(tc.tile_pool(name="psb", bufs=1, space="PSUM"))
        pss = ctx.enter_context(tc.tile_pool(name="pss", bufs=2, space="PSUM"))

        # ---- DRAM I/O ----
        x_d = dram.tile([N, 3], F32, kind="ExternalInput", uniquify=False, name="x")
        win = {}
        for li, (C, O) in enumerate(LAYERS, start=1):
            win[f'wnt{li}'] = dram.tile([C, O], F32, kind="ExternalInput", uniquify=False, name=f"wnt{li}")
            win[f'wdt{li}'] = dram.tile([C, O], F32, kind="ExternalInput", uniquify=False, name=f"wdt{li}")
            win[f'bs{li}'] = dram.tile([min(O, 128), max(1, O // 128)], F32, kind="ExternalInput", uniquify=False, name=f"bs{li}")
        w5t_d = dram.tile([512, 512], F32, kind="ExternalInput", uniquify=False, name="w5t")
        b5_d = dram.tile([1, 512], F32, kind="ExternalInput", uniquify=False, name="b5")
        wfc1_d = dram.tile([512, 256], F32, kind="ExternalInput", uniquify=False, name="wfc1")
        bfc1_d = dram.tile([128, 2], F32, kind="ExternalInput", uniquify=False, name="bfc1")
        wfc2_d = dram.tile([256, 128], F32, kind="ExternalInput", uniquify=False, name="wfc2")
        bfc2_d = dram.tile([128, 1], F32, kind="ExternalInput", uniquify=False, name="bfc2")
        wfc3_d = dram.tile([128, 40], F32, kind="ExternalInput", uniquify=False, name="wfc3")
        bfc3_d = dram.tile([1, 40], F32, kind="ExternalInput", uniquify=False, name="bfc3")
        out_d = dram.tile([40, 1], F32, kind="ExternalOutput", uniquify=False, name="out")

        gdts = {1: F32, 2: F32, 3: F16, 4: F16}
        a_ds = {li: dram.tile([N, O], gdts[li], name=f"a_d{li}")
                for li, (C, O) in enumerate(LAYERS, start=1)}
        jw_ds = {li: dram.tile([N * KNN // 16, 128], I16, name=f"jw_d{li}")
                 for li in range(1, 5)}

        # ---- consts ----
        iotaJ = consts.tile([128, N], U32, tag="iotaJ")
        nc.gpsimd.iota(iotaJ[:, :], [[1, N]], base=0, channel_multiplier=0)
        ident = consts.tile([128, 128], F32, tag="ident")
        make_identity(nc, ident[:, :])
        onescol = consts.tile([128, 1], F32, tag="onescol")
        nc.vector.memset(onescol[:, :], 1.0)
        onescolR = consts.tile([128, 1], F32, tag="onescolR")
        nc.sync.dma_start(_r(onescolR[:, :]), _r(onescol[:, :]))
        onesrow = consts.tile([1, N], F32, tag="onesrow")
        nc.vector.memset(onesrow[:, :], 1.0)
        onesrowR = consts.tile([1, N], F32, tag="onesrowR")
        nc.sync.dma_start(_r(onesrowR[:, :]), _r(onesrow[:, :]))

        # persistent feature tensors (augmented with a trailing ones row
        # where the next layer uses the ones-trick, i.e. C_next + 1 <= 128)
        x0T = persist.tile([4, N], F32, tag="x0T")
        x1T = persist.tile([65, N], F32, tag="x1T")
        x2T = persist.tile([65, N], F32, tag="x2T")
        x3T = persist.tile([128, N], F32, tag="x3T")
        x4Ta = persist.tile([128, N], F32, tag="x4Ta")
        x4Tb = persist.tile([128, N], F32, tag="x4Tb")
        nc.sync.dma_start(_r(x0T[3:4, :]), _r(onesrow[0:1, :]))
        nc.sync.dma_start(_r(x1T[64:65, :]), _r(onesrow[0:1, :]))
        nc.sync.dma_start(_r(x2T[64:65, :]), _r(onesrow[0:1, :]))

        # load x transposed: x_d is (N, 3) row-major
        xap = x_d[:, :]
        nc.sync.dma_start(
            _r(x0T[0:3, :]),
            bass.AP(xap.tensor, xap.offset, [[1, 3], [3, N]]).bitcast(F32R))

        def edge_layer(li, xT, C, O, out_parts, post_tiles=None,
                       post_half=None):
            """xT: [C(+1), N] features (row C = ones iff aug). out_parts:
            list of (dest_ap, orow, ocol0)."""
            aug = (C + 1 <= 128) and li < 4
            gdt = gdts[li]
            a_d = a_ds[li]
            jw_d = jw_ds[li]
            jwap = jw_d[:, :]
            wnt = sb.tile([C, O], F32, tag="wnt")
            wdt = sb.tile([C, O], F32, tag="wdt")
            bs = sb.tile([min(O, 128), max(1, O // 128)], F32, tag="bs")
            nc.sync.dma_start(_r(wnt[:, :]), _r(win[f'wnt{li}'][:, :]))
            nc.sync.dma_start(_r(wdt[:, :]), _r(win[f'wdt{li}'][:, :]))
            nc.sync.dma_start(bs[:, :], win[f'bs{li}'][:, :])

            xsq = sb.tile([C, N], F32, tag="xsq")
            if aug:
                x2dA = sb.tile([C + 1, N], F32, tag="x2dA")
            else:
                x2dA = sb.tile([C, N], F32, tag="x2dA")
            # compute engines may only start at partition 0/32/64/96: stage
            # negS in a [1, N] tile and DMA into the augmented row otherwise
            direct = aug and C % 32 == 0
            negS = None if direct else sb.tile([1, N], F32, tag="negS")
            for h in range(2):
                cols = slice(h * 512, (h + 1) * 512)
                nc.scalar.activation(_r(xsq[:, cols]), xT[0:C, cols], AF.Square)
                nc.scalar.activation(
                    _r(x2dA[0:C, cols]), xT[0:C, cols], AF.Copy, bias=0.0,
                    scale=2.0)
                S_ps = pss.tile([1, 512], F32, tag="a")
                nc.tensor.matmul(
                    S_ps[:, :], lhsT=_r(onescolR[0:C, :]), rhs=_r(xsq[:, cols]),
                    start=True, stop=True, skip_group_check=True)
                dst = x2dA[C:C + 1, cols] if direct else negS[0:1, cols]
                nc.scalar.activation(_r(dst), S_ps[:, :], AF.Copy, bias=0.0,
                                     scale=-1.0)
                if aug and not direct:
                    nc.sync.dma_start(_r(x2dA[C:C + 1, cols]), _r(negS[0:1, cols]))

            negSc = sb.tile([128, NT], F32, tag="negSc")

            def emit_negSc(t):
                nsp = pss.tile([128, 1], F32, tag="a")
                nc.tensor.matmul(
                    nsp[:, :], lhsT=xsq[:, t * 128:(t + 1) * 128],
                    rhs=onescol[0:C, :],
                    start=True, stop=True, skip_group_check=True)
                nc.scalar.activation(
                    negSc[:, t:t + 1], nsp[:, :], AF.Copy, bias=0.0, scale=-1.0)

            # px PSUM tiles; c-part matmuls issued early (start=True)
            pxs = []
            for (dst_ap, orow, oc0) in out_parts:
                px = psb.tile([orow, N], F32, tag=f"px{oc0}")
                pxs.append(px)

            m = mp.tile([128, NT, O], F32, tag=f"m{li}")

            state = {}

            def emit_a_rows():
                # block a-tiles into [128, 512]-col PSUM groups: one eviction
                # and one (3-level AP) DMA per group instead of per tile
                tpg = 512 // O                      # tiles per group
                for g0 in range(0, NT, tpg):
                    ng = min(tpg, NT - g0)
                    a_ps = pss.tile([128, ng, O], F32, tag="a")
                    for ti in range(ng):
                        lt = xT[0:C, (g0 + ti) * 128:(g0 + ti + 1) * 128]
                        wv = wnt[:, :]
                        if O >= 256:
                            lt, wv = _r(lt), _r(wv)
                        nc.tensor.matmul(
                            a_ps[:, ti, :], lhsT=lt, rhs=wv,
                            start=True, stop=True, skip_group_check=True)
                    a_sb = sb.tile([128, ng, O], gdt, tag="a_sb")
                    nc.scalar.activation(
                        a_sb[:, :, :].rearrange("p t o -> p (t o)"),
                        a_ps[:, :, :].rearrange("p t o -> p (t o)"), AF.Copy)
                    adap = a_d[:, :]
                    dst = bass.AP(adap.tensor, adap.offset + g0 * 128 * O,
                                  [[O, 128], [128 * O, ng], [1, O]])
                    nc.sync.dma_start(dst, a_sb[:, :, :])

            def emit_c_parts():
                for pi, (dst_t, orow, oc0) in enumerate(out_parts):
                    for h in range(2):
                        cols = slice(h * 512, (h + 1) * 512)
                        nc.tensor.matmul(
                            pxs[pi][:, cols],
                            lhsT=_r(wdt[:, oc0:oc0 + orow]), rhs=_r(xT[0:C, cols]),
                            start=True, stop=False, skip_group_check=True)

            def emit_kp(t):
                tcols = slice(t * 128, (t + 1) * 128)
                kp = psb.tile([128, N], F32, tag="kp")
                for h in range(2):
                    cols = slice(h * 512, (h + 1) * 512)
                    if aug:
                        nc.tensor.matmul(
                            kp[:, cols], lhsT=_r(xT[:, tcols]), rhs=_r(x2dA[:, cols]),
                            start=True, stop=True, skip_group_check=True)
                    else:
                        nc.tensor.matmul(
                            kp[:, cols], lhsT=_r(xT[0:C, tcols]), rhs=_r(x2dA[:, cols]),
                            start=True, stop=False, skip_group_check=True)
                        nc.tensor.matmul(
                            kp[:, cols], lhsT=_r(onesrowR[0:1, tcols]),
                            rhs=_r(negS[0:1, cols]),
                            start=False, stop=True, skip_group_check=True)
                kpc = kpcp.tile([128, N], F32, tag="kpc")
                nc.scalar.activation(
                    kpc[:, :], kp[:, :], AF.Prelu, bias=negSc[:, t:t + 1],
                    scale=1.0, alpha=1.0)
                kb = keyp.tile([128, N], U32, tag="kb")
                nc.vector.tensor_scalar(
                    kb[:, :], kpc[:, :].bitcast(U32), 0xFFFFFC00, None,
                    op0=ALU.bitwise_and)
                nc.vector.tensor_tensor(
                    out=kb[:, :], in0=kb[:, :], in1=iotaJ[:, :], op=ALU.bitwise_or)
                state[t] = {'kb': kb}

            def emit_topk(t):
                kbf = state[t]['kb'][:, :].bitcast(F32)
                v24 = smalls.tile([128, 24], F32, tag="v24")
                nc.vector.max(v24[:, 0:8], kbf)
                nc.vector.match_replace(kbf, v24[:, 0:8], kbf, NEG_BIG)
                nc.vector.max(v24[:, 8:16], kbf)
                nc.vector.match_replace(kbf, v24[:, 8:16], kbf, NEG_BIG)
                nc.vector.max(v24[:, 16:24], kbf)
                j20 = smalls.tile([128, KNN], U32, tag="j20")
                nc.vector.tensor_scalar(
                    j20[:, :], v24[:, 0:KNN].bitcast(U32), 0x3FF, None,
                    op0=ALU.bitwise_and)
                jf20 = smalls.tile([128, KNN], F32, tag="jf20")
                nc.vector.tensor_copy(jf20[:, :], j20[:, :])
                state[t]['jf20'] = jf20

            def emit_idq(t):
                jT_ps = pss.tile([KNN, 128], F32, tag="a")
                nc.tensor.matmul(
                    jT_ps[:, :], lhsT=state[t]['jf20'][:, :], rhs=ident[:, 0:128],
                    is_transpose=True, start=True, stop=True, skip_group_check=True)
                jTi = smalls.tile([KNN, 128], I16, tag="jTi")
                nc.scalar.activation(jTi[:, :], jT_ps[:, :], AF.Copy)
                dst = bass.AP(jwap.tensor, jwap.offset + t * 160 * 128,
                              [[1024, KNN], [128, 8], [1, 16]])
                nc.sync.dma_start(
                    dst, jTi[:, :].rearrange("k (h s) -> k h s", s=16))
                src_ap = bass.AP(jwap.tensor, jwap.offset + t * 160 * 128,
                                 [[128, 160], [1, 128]])
                idq = idqp.tile([128, 160], I16, tag="idq")
                nc.sync.dma_start_transpose(idq[:, :], src_ap)
                nc.sync.dma_start(idq[16:32, :], idq[0:16, :])
                nc.sync.dma_start(idq[32:64, :], idq[0:32, :])
                nc.sync.dma_start(idq[64:128, :], idq[0:64, :])
                state[t]['idq'] = idq

            def emit_gathers(t):
                g = gath.tile([128, KNN, O], gdt, tag="g")
                idq = state[t]['idq']
                for q in range(NQ):
                    nc.gpsimd.dma_gather(
                        out_ap=g[:, 5 * q:5 * q + 5, :], in_ap=a_d[:, :],
                        idxs_ap=idq[:, 40 * q:40 * (q + 1)],
                        num_idxs=5 * 128, num_idxs_reg=5 * 128, elem_size=O,
                        single_packet=False, queue_num=q)
                state[t]['g'] = g

            def emit_reduce(t):
                # contiguous max tree over k=20: 10+10 -> 5 -> (2+2)+1 -> 1
                g = state[t]['g']
                r = gath.tile([128, 10, O], gdt, tag="r", bufs=2)
                nc.vector.tensor_tensor(
                    out=r[:, :, :], in0=g[:, 0:10, :], in1=g[:, 10:20, :],
                    op=ALU.max)
                nc.vector.tensor_tensor(
                    out=r[:, 0:5, :], in0=r[:, 0:5, :], in1=r[:, 5:10, :],
                    op=ALU.max)
                nc.vector.tensor_tensor(
                    out=r[:, 0:2, :], in0=r[:, 0:2, :], in1=r[:, 2:4, :],
                    op=ALU.max)
                nc.vector.tensor_tensor(
                    out=r[:, 0, :], in0=r[:, 0, :], in1=r[:, 1, :], op=ALU.max)
                nc.vector.tensor_tensor(
                    out=m[:, t, :], in0=r[:, 0, :], in1=r[:, 4, :], op=ALU.max)
                for pi, (dst_t, orow, oc0) in enumerate(out_parts):
                    nc.tensor.matmul(
                        pxs[pi][:, t * 128:(t + 1) * 128],
                        lhsT=m[:, t, oc0:oc0 + orow], rhs=ident[:, 0:128],
                        is_transpose=True, start=False, stop=(t % 4 == 3),
                        skip_group_check=True)
                state[t] = None  # release refs

            def emit_out_half(h):
                # Prelu with per-partition bias folds the bias matmul; the px
                # bank is finalized by its last transpose (stop at t%4==3)
                cols = slice(h * 512, (h + 1) * 512)
                for pi, (dst_t, orow, oc0) in enumerate(out_parts):
                    nc.scalar.activation(
                        _r(dst_t[0:orow, cols]), pxs[pi][:, cols], AF.Prelu,
                        bias=bs[0:orow, pi:pi + 1], scale=1.0, alpha=NEG_SLOPE)

            # ---- pipelined tile loop ----
            for t in range(NT):
                emit_negSc(t)
                emit_kp(t)
                if t == 0:
                    emit_a_rows()
                if t == 1:
                    emit_c_parts()
                if t == 2 and post_tiles is not None:
                    post_tiles()
                if t >= 1:
                    emit_idq(t - 1)
                    emit_gathers(t - 1)
                emit_topk(t)
                if t >= 3:
                    emit_reduce(t - 3)
            emit_idq(NT - 1)
            emit_gathers(NT - 1)
            for t in range(NT - 3, NT):
                emit_reduce(t)
                if t == NT - 3:
                    emit_out_half(0)
                    if post_half is not None:
                        post_half(0)
            emit_out_half(1)
            if post_half is not None:
                post_half(1)

        # ---- conv5 weights staged early ----
        w5sb = {}
        for ci, (rows, k0) in enumerate([(64, 0), (64, 64), (128, 128),
                                         (128, 256), (128, 384)]):
            w5c = consts.tile([rows, 512], F32, tag=f"w5c{ci}")
            nc.sync.dma_start(_r(w5c[:, :]), _r(w5t_d[k0:k0 + rows, :]))
            w5sb[ci] = w5c
        b5sb = consts.tile([1, 512], F32, tag="b5sb")
        nc.sync.dma_start(b5sb[:, :], b5_d[:, :])
        zpart = persist.tile([128, NT, 512], F32, tag="zpart")
        zp12 = persist.tile([128, NT, 512], F32, tag="zp12")

        def zp12_fill():
            for t in range(NT):
                tcols = slice(t * 128, (t + 1) * 128)
                zp_ps = pss.tile([128, 512], F32, tag="a")
                for ci, (xt, rows) in enumerate([(x1T, 64), (x2T, 64)]):
                    nc.tensor.matmul(
                        zp_ps[:, :], lhsT=_r(xt[0:rows, tcols]),
                        rhs=_r(w5sb[ci][:, :]),
                        start=(ci == 0), stop=(ci == 1), skip_group_check=True)
                nc.scalar.activation(_r(zp12[:, t, :]), zp_ps[:, :], AF.Copy)

        def zpart_fill():
            for t in range(NT):
                tcols = slice(t * 128, (t + 1) * 128)
                zp_ps = pss.tile([128, 512], F32, tag="a")
                nc.tensor.matmul(
                    zp_ps[:, :], lhsT=_r(x3T[:, tcols]), rhs=_r(w5sb[2][:, :]),
                    start=True, stop=False, skip_group_check=True)
                nc.tensor.matmul(
                    zp_ps[:, :], lhsT=ident[:, 0:128], rhs=zp12[:, t, :],
                    start=False, stop=True, skip_group_check=True)
                nc.scalar.activation(zpart[:, t, :], zp_ps[:, :], AF.Copy)

        edge_layer(1, x0T, 3, 64, [(x1T, 64, 0)])
        edge_layer(2, x1T, 64, 64, [(x2T, 64, 0)])
        edge_layer(3, x2T, 64, 128, [(x3T, 128, 0)], post_tiles=zp12_fill)
        edge_layer(4, x3T, 128, 256, [(x4Ta, 128, 0), (x4Tb, 128, 128)],
                   post_tiles=zpart_fill)

        # ---- head: conv5 (x4 chunks; x1-x3 partials precomputed) + max pool ----
        zmax = persist.tile([128, 512], F32, tag="zmax")
        for t in range(NT):
            tcols = slice(t * 128, (t + 1) * 128)
            z_ps = pss.tile([128, 512], F32, tag="a")
            for ci, (xt, k0) in enumerate([(x4Ta, 256), (x4Tb, 384)]):
                nc.tensor.matmul(
                    z_ps[:, :], lhsT=_r(xt[:, tcols]), rhs=_r(w5sb[3 + ci][:, :]),
                    start=(ci == 0), stop=False, skip_group_check=True)
            nc.tensor.matmul(
                z_ps[:, :], lhsT=onesrow[0:1, tcols],
                rhs=b5sb[:, :], start=False, stop=True, skip_group_check=True)
            zsb = sb.tile([128, 512], F32, tag="zsb")
            nc.vector.tensor_tensor(
                out=zsb[:, :], in0=zpart[:, t, :], in1=z_ps[:, :], op=ALU.add)
            if t == 0:
                nc.scalar.activation(zmax[:, :], zsb[:, :], AF.Copy)
            else:
                nc.vector.tensor_tensor(
                    out=zmax[:, :], in0=zmax[:, :], in1=zsb[:, :], op=ALU.max)
        # transpose zmax chunks and reduce along free dim -> yT [128, 4]
        yT = persist.tile([128, 4], F32, tag="yT")
        for cchunk in range(4):
            zt_ps = pss.tile([128, 128], F32, tag="a")
            nc.tensor.matmul(
                zt_ps[:, :], lhsT=zmax[:, cchunk * 128:(cchunk + 1) * 128],
                rhs=ident[:, 0:128], is_transpose=True, start=True, stop=True,
                skip_group_check=True)
            nc.vector.tensor_reduce(
                out=yT[:, cchunk:cchunk + 1], in_=zt_ps[:, :],
                axis=AX.X, op=ALU.max)
        yTr = persist.tile([128, 4], F32, tag="yTr")
        nc.scalar.activation(yTr[:, :], yT[:, :], AF.Prelu, alpha=NEG_SLOPE)

        # ---- FC head ----
        wfc1sb = consts.tile([128, 4, 256], F32, tag="wfc1sb")
        for c in range(4):
            nc.sync.dma_start(wfc1sb[:, c, :], wfc1_d[c * 128:(c + 1) * 128, :])
        bfc1sb = consts.tile([128, 2], F32, tag="bfc1sb")
        nc.sync.dma_start(bfc1sb[:, :], bfc1_d[:, :])
        wfc2sb = consts.tile([128, 2, 128], F32, tag="wfc2sb")
        for c in range(2):
            nc.sync.dma_start(wfc2sb[:, c, :], wfc2_d[c * 128:(c + 1) * 128, :])
        bfc2sb = consts.tile([128, 1], F32, tag="bfc2sb")
        nc.sync.dma_start(bfc2sb[:, :], bfc2_d[:, :])
        wfc3sb = consts.tile([128, 40], F32, tag="wfc3sb")
        nc.sync.dma_start(wfc3sb[:, :], wfc3_d[:, :])
        bfc3sb = consts.tile([1, 40], F32, tag="bfc3sb")
        nc.sync.dma_start(bfc3sb[:, :], bfc3_d[:, :])

        h1sb = persist.tile([128, 2], F32, tag="h1sb")
        for mt in range(2):
            h1_ps = pss.tile([128, 1], F32, tag="a")
            for c in range(4):
                nc.tensor.matmul(
                    h1_ps[:, :], lhsT=wfc1sb[:, c, mt * 128:(mt + 1) * 128],
                    rhs=yTr[:, c:c + 1],
                    start=(c == 0), stop=(c == 3), skip_group_check=True)
            nc.scalar.activation(
                h1sb[:, mt:mt + 1], h1_ps[:, :], AF.Prelu,
                bias=bfc1sb[:, mt:mt + 1], scale=1.0, alpha=NEG_SLOPE)
        h2sb = persist.tile([128, 1], F32, tag="h2sb")
        h2_ps = pss.tile([128, 1], F32, tag="a")
        for c in range(2):
            nc.tensor.matmul(
                h2_ps[:, :], lhsT=wfc2sb[:, c, :], rhs=h1sb[:, c:c + 1],
                start=(c == 0), stop=(c == 1), skip_group_check=True)
        nc.scalar.activation(
            h2sb[:, :], h2_ps[:, :], AF.Prelu,
            bias=bfc2sb[:, :], scale=1.0, alpha=NEG_SLOPE)

        out_ps = pss.tile([40, 1], F32, tag="a")
        nc.tensor.matmul(
            out_ps[:, :], lhsT=wfc3sb[:, :], rhs=h2sb[:, :],
            start=True, stop=False, skip_group_check=True)
        nc.tensor.matmul(
            out_ps[:, :], lhsT=bfc3sb[:, :], rhs=onescol[0:1, :],
            start=False, stop=True, skip_group_check=True)
        out_sb = persist.tile([40, 1], F32, tag="out_sb")
        nc.scalar.activation(out_sb[:, :], out_ps[:, :], AF.Copy)
        nc.sync.dma_start(out_d[:, :], out_sb[:, :])


# ---------------------------------------------------------------------------
# harness entry point
# ---------------------------------------------------------------------------
_NC_CACHE = {}


def _get_nc():
    if 'nc' not in _NC_CACHE:
        _NC_CACHE['nc'] = build_nc()
    return _NC_CACHE['nc']


def kernel(**inputs):
    """Full-batch EdgeCNN forward. x: (8, 1024, 3) -> (8, 40) float32.

    Pure data parallel: batch element b runs on NeuronCore b.
    """
    from concourse.bass_utils import run_bass_kernel_spmd

    inp = {k: np.asarray(v) for k, v in inputs.items()}
    prep = host_prep(inp)
    nc = _get_nc()
    in_maps = []
    for b in range(8):
        m = {'x': np.ascontiguousarray(inp['x'][b]).astype(np.float32)}
        m.update(prep)
        in_maps.append(m)
    res = run_bass_kernel_spmd(nc, in_maps, core_ids=list(range(8)))
    out = np.stack([res.results[b]['out'].reshape(40) for b in range(8)])
    return out.astype(np.float32)


# revision 29
# speedup vs baseline: 1.3550x; 1.1900x over previous
"""EdgeCNN (DGCNN) Bass/Tile kernel for TRN2 — one batch element per core.

Per edge-conv layer (N=1024 points, K=20 neighbors):
  1. PE: augmented matmul key[n,j] = 2<xn,xj> - S[j]  (ones-row trick folds
     the -S[j] column term; the -S[n] row term is rank-invariant and dropped)
  2. ACT: evict keys PSUM -> SBUF
  3. GPSIMD: AND low-10 mantissa bits, OR in column index j -> packed keys
  4. DVE: 3x max8 + 2x match_replace -> top-20 packed keys; extract j
  5. idx -> DRAM -> read back wrapped (partition = i%16); dma_gather of
     a = x @ (g~ Wn)^T rows SPLIT 4-WAY across SWDGE queues 0-3 (concurrent
     Q7 cpu pairs + DMA paths)
  6. DVE: strided reduce_max over k; PE: transpose(m) + c-matmul accumulate
  7. ACT: leaky-relu (Prelu alpha=0.2) PSUM -> next layer xT
Head: conv5 via K-chunk accumulation, global max-pool, 3 FC layers on PE.
"""

import contextlib

import numpy as np

import concourse.bass as bass
import concourse.bacc as bacc
import concourse.mybir as mybir
from concourse.tile import TileContext
from concourse.masks import make_identity

F32 = mybir.dt.float32
U32 = mybir.dt.uint32
I16 = mybir.dt.int16
F16 = mybir.dt.float16
F32R = mybir.dt.float32r


def _r(ap):
    return ap.bitcast(F32R)
AF = mybir.ActivationFunctionType
ALU = mybir.AluOpType
AX = mybir.AxisListType

N = 1024
KNN = 20
NT = 8
NQ = 4            # SWDGE queues
NEG_SLOPE = 0.2
BNI = np.float32(1.0 / np.sqrt(1.0 + 1e-5))
LAYERS = [(3, 64), (64, 64), (64, 128), (128, 256)]
NEG_BIG = -3.0e38


def host_prep(inp):
    """Fold BN scale/bias into weights; transpose for device layout."""
    d = {}
    for li, (C, O) in enumerate(LAYERS, start=1):
        W = inp[f'W{li}'].astype(np.float32)
        g = inp[f'g{li}'].astype(np.float32)
        b = inp[f'b{li}'].astype(np.float32)
        gt = g * BNI
        Wn = W[:, :C]
        Wc = W[:, C:]
        d[f'wnt{li}'] = np.ascontiguousarray((gt[:, None] * Wn).T)          # (C, O)
        d[f'wdt{li}'] = np.ascontiguousarray((gt[:, None] * (Wc - Wn)).T)   # (C, O)
        d[f'bs{li}'] = np.ascontiguousarray(b.reshape(max(1, O // 128), min(O, 128)).T)
    g5 = inp['g5'].astype(np.float32) * BNI
    d['w5t'] = np.ascontiguousarray((g5[:, None] * inp['W5']).T)            # (512, 512)
    d['b5'] = inp['b5'].reshape(1, 512).astype(np.float32).copy()
    g1 = inp['bng1'].astype(np.float32) * BNI
    d['wfc1'] = np.ascontiguousarray((g1[:, None] * inp['fc1_w']).T)        # (512, 256)
    bf1 = g1 * inp['fc1_b'].astype(np.float32) + inp['bnb1'].astype(np.float32)
    d['bfc1'] = np.ascontiguousarray(bf1.reshape(2, 128).T)                 # (128, 2)
    g2 = inp['bng2'].astype(np.float32) * BNI
    d['wfc2'] = np.ascontiguousarray((g2[:, None] * inp['fc2_w']).T)        # (256, 128)
    bf2 = g2 * inp['fc2_b'].astype(np.float32) + inp['bnb2'].astype(np.float32)
    d['bfc2'] = np.ascontiguousarray(bf2.reshape(128, 1))                   # (128, 1)
    d['wfc3'] = np.ascontiguousarray(inp['fc3_w'].T)                        # (128, 40)
    d['bfc3'] = inp['fc3_b'].reshape(1, 40).astype(np.float32).copy()
    return d


def build_nc():
    nc = bacc.Bacc("TRN2", target_bir_lowering=False, debug=False, num_devices=8,
                   num_swdge_queues=NQ)
    with TileContext(nc) as tc:
        _trace(nc, tc)
    nc.compile()
    return nc


def _trace(nc, tc):
    with contextlib.ExitStack() as ctx:
        dram = ctx.enter_context(tc.tile_pool(name="dram", bufs=1, space="DRAM"))
        consts = ctx.enter_context(tc.tile_pool(name="consts", bufs=1))
        persist = ctx.enter_context(tc.tile_pool(name="persist", bufs=1))
        sb = ctx.enter_context(tc.tile_pool(name="sb", bufs=2))
        kpcp = ctx.enter_context(tc.tile_pool(name="kpcp", bufs=2))
        keyp = ctx.enter_context(tc.tile_pool(name="keyp", bufs=2))
        smalls = ctx.enter_context(tc.tile_pool(name="smalls", bufs=4))
        idqp = ctx.enter_context(tc.tile_pool(name="idqp", bufs=4))
        gath = ctx.enter_context(tc.tile_pool(name="gath", bufs=3))
        mp = ctx.enter_context(tc.tile_pool(name="mp", bufs=1))
        psb = ctx.enter_context(tc.tile_pool(name="psb", bufs=1, space="PSUM"))
        pss = ctx.enter_context(tc.tile_pool(name="pss", bufs=2, space="PSUM"))

        # ---- DRAM I/O ----
        x_d = dram.tile([N, 3], F32, kind="ExternalInput", uniquify=False, name="x")
        win = {}
        for li, (C, O) in enumerate(LAYERS, start=1):
            win[f'wnt{li}'] = dram.tile([C, O], F32, kind="ExternalInput", uniquify=False, name=f"wnt{li}")
            win[f'wdt{li}'] = dram.tile([C, O], F32, kind="ExternalInput", uniquify=False, name=f"wdt{li}")
            win[f'bs{li}'] = dram.tile([min(O, 128), max(1, O // 128)], F32, kind="ExternalInput", uniquify=False, name=f"bs{li}")
        w5t_d = dram.tile([512, 512], F32, kind="ExternalInput", uniquify=False, name="w5t")
        b5_d = dram.tile([1, 512], F32, kind="ExternalInput", uniquify=False, name="b5")
        wfc1_d = dram.tile([512, 256], F32, kind="ExternalInput", uniquify=False, name="wfc1")
        bfc1_d = dram.tile([128, 2], F32, kind="ExternalInput", uniquify=False, name="bfc1")
        wfc2_d = dram.tile([256, 128], F32, kind="ExternalInput", uniquify=False, name="wfc2")
        bfc2_d = dram.tile([128, 1], F32, kind="ExternalInput", uniquify=False, name="bfc2")
        wfc3_d = dram.tile([128, 40], F32, kind="ExternalInput", uniquify=False, name="wfc3")
        bfc3_d = dram.tile([1, 40], F32, kind="ExternalInput", uniquify=False, name="bfc3")
        out_d = dram.tile([40, 1], F32, kind="ExternalOutput", uniquify=False, name="out")

        gdts = {1: F32, 2: F32, 3: F16, 4: F16}
        a_ds = {li: dram.tile([N, O], gdts[li], name=f"a_d{li}")
                for li, (C, O) in enumerate(LAYERS, start=1)}
        jw_ds = {li: dram.tile([N * KNN // 16, 128], I16, name=f"jw_d{li}")
                 for li in range(1, 5)}

        # ---- consts ----
        iotaJ = consts.tile([128, N], U32, tag="iotaJ")
        nc.gpsimd.iota(iotaJ[:, :], [[1, N]], base=0, channel_multiplier=0)
        ident = consts.tile([128, 128], F32, tag="ident")
        make_identity(nc, ident[:, :])
        onescol = consts.tile([128, 1], F32, tag="onescol")
        nc.vector.memset(onescol[:, :], 1.0)
        onescolR = consts.tile([128, 1], F32, tag="onescolR")
        nc.sync.dma_start(_r(onescolR[:, :]), _r(onescol[:, :]))
        onesrow = consts.tile([1, N], F32, tag="onesrow")
        nc.vector.memset(onesrow[:, :], 1.0)
        onesrowR = consts.tile([1, N], F32, tag="onesrowR")
        nc.sync.dma_start(_r(onesrowR[:, :]), _r(onesrow[:, :]))

        # persistent feature tensors (augmented with a trailing ones row
        # where the next layer uses the ones-trick, i.e. C_next + 1 <= 128)
        x0T = persist.tile([4, N], F32, tag="x0T")
        x1T = persist.tile([65, N], F32, tag="x1T")
        x2T = persist.tile([65, N], F32, tag="x2T")
        x3T = persist.tile([128, N], F32, tag="x3T")
        x4Ta = persist.tile([128, N], F32, tag="x4Ta")
        x4Tb = persist.tile([128, N], F32, tag="x4Tb")
        nc.sync.dma_start(_r(x0T[3:4, :]), _r(onesrow[0:1, :]))
        nc.sync.dma_start(_r(x1T[64:65, :]), _r(onesrow[0:1, :]))
        nc.sync.dma_start(_r(x2T[64:65, :]), _r(onesrow[0:1, :]))

        # load x transposed: x_d is (N, 3) row-major
        xap = x_d[:, :]
        nc.sync.dma_start(
            _r(x0T[0:3, :]),
            bass.AP(xap.tensor, xap.offset, [[1, 3], [3, N]]).bitcast(F32R))

        def edge_layer(li, xT, C, O, out_parts, post_tiles=None,
                       post_half=None):
            """xT: [C(+1), N] features (row C = ones iff aug). out_parts:
            list of (dest_ap, orow, ocol0)."""
            aug = (C + 1 <= 128) and li < 4
            gdt = gdts[li]
            a_d = a_ds[li]
            jw_d = jw_ds[li]
            jwap = jw_d[:, :]
            wnt = sb.tile([C, O], F32, tag="wnt")
            wdt = sb.tile([C, O], F32, tag="wdt")
            bs = sb.tile([min(O, 128), max(1, O // 128)], F32, tag="bs")
            nc.sync.dma_start(_r(wnt[:, :]), _r(win[f'wnt{li}'][:, :]))
            nc.sync.dma_start(_r(wdt[:, :]), _r(win[f'wdt{li}'][:, :]))
            nc.sync.dma_start(bs[:, :], win[f'bs{li}'][:, :])

            xsq = sb.tile([C, N], F32, tag="xsq")
            if aug:
                x2dA = sb.tile([C + 1, N], F32, tag="x2dA")
            else:
                x2dA = sb.tile([C, N], F32, tag="x2dA")
            # compute engines may only start at partition 0/32/64/96: stage
            # negS in a [1, N] tile and DMA into the augmented row otherwise
            direct = aug and C % 32 == 0
            negS = None if direct else sb.tile([1, N], F32, tag="negS")
            for h in range(2):
                cols = slice(h * 512, (h + 1) * 512)
                nc.scalar.activation(_r(xsq[:, cols]), xT[0:C, cols], AF.Square)
                nc.scalar.activation(
                    _r(x2dA[0:C, cols]), xT[0:C, cols], AF.Copy, bias=0.0,
                    scale=2.0)
                S_ps = pss.tile([1, 512], F32, tag="a")
                nc.tensor.matmul(
                    S_ps[:, :], lhsT=_r(onescolR[0:C, :]), rhs=_r(xsq[:, cols]),
                    start=True, stop=True, skip_group_check=True)
                dst = x2dA[C:C + 1, cols] if direct else negS[0:1, cols]
                nc.scalar.activation(_r(dst), S_ps[:, :], AF.Copy, bias=0.0,
                                     scale=-1.0)
                if aug and not direct:
                    nc.sync.dma_start(_r(x2dA[C:C + 1, cols]), _r(negS[0:1, cols]))

            negSc = sb.tile([128, NT], F32, tag="negSc")

            def emit_negSc(t):
                nsp = pss.tile([128, 1], F32, tag="a")
                nc.tensor.matmul(
                    nsp[:, :], lhsT=xsq[:, t * 128:(t + 1) * 128],
                    rhs=onescol[0:C, :],
                    start=True, stop=True, skip_group_check=True)
                nc.scalar.activation(
                    negSc[:, t:t + 1], nsp[:, :], AF.Copy, bias=0.0, scale=-1.0)

            # px PSUM tiles; c-part matmuls issued early (start=True)
            pxs = []
            for (dst_ap, orow, oc0) in out_parts:
                px = psb.tile([orow, N], F32, tag=f"px{oc0}")
                pxs.append(px)

            m = mp.tile([128, NT, O], F32, tag=f"m{li}")

            state = {}

            def emit_a_rows():
                # block a-tiles into [128, 512]-col PSUM groups: one eviction
                # and one (3-level AP) DMA per group instead of per tile
                tpg = 512 // O                      # tiles per group
                for g0 in range(0, NT, tpg):
                    ng = min(tpg, NT - g0)
                    a_ps = pss.tile([128, ng, O], F32, tag="a")
                    for ti in range(ng):
                        lt = xT[0:C, (g0 + ti) * 128:(g0 + ti + 1) * 128]
                        wv = wnt[:, :]
                        if O >= 256:
                            lt, wv = _r(lt), _r(wv)
                        nc.tensor.matmul(
                            a_ps[:, ti, :], lhsT=lt, rhs=wv,
                            start=True, stop=True, skip_group_check=True)
                    a_sb = sb.tile([128, ng, O], gdt, tag="a_sb")
                    nc.scalar.activation(
                        a_sb[:, :, :].rearrange("p t o -> p (t o)"),
                        a_ps[:, :, :].rearrange("p t o -> p (t o)"), AF.Copy)
                    adap = a_d[:, :]
                    dst = bass.AP(adap.tensor, adap.offset + g0 * 128 * O,
                                  [[O, 128], [128 * O, ng], [1, O]])
                    nc.sync.dma_start(dst, a_sb[:, :, :])

            def emit_c_parts():
                for pi, (dst_t, orow, oc0) in enumerate(out_parts):
                    for h in range(2):
                        cols = slice(h * 512, (h + 1) * 512)
                        nc.tensor.matmul(
                            pxs[pi][:, cols],
                            lhsT=_r(wdt[:, oc0:oc0 + orow]), rhs=_r(xT[0:C, cols]),
                            start=True, stop=False, skip_group_check=True)

            def emit_kp(t):
                tcols = slice(t * 128, (t + 1) * 128)
                kp = psb.tile([128, N], F32, tag="kp")
                for h in range(2):
                    cols = slice(h * 512, (h + 1) * 512)
                    if aug:
                        nc.tensor.matmul(
                            kp[:, cols], lhsT=_r(xT[:, tcols]), rhs=_r(x2dA[:, cols]),
                            start=True, stop=True, skip_group_check=True)
                    else:
                        nc.tensor.matmul(
                            kp[:, cols], lhsT=_r(xT[0:C, tcols]), rhs=_r(x2dA[:, cols]),
                            start=True, stop=False, skip_group_check=True)
                        nc.tensor.matmul(
                            kp[:, cols], lhsT=_r(onesrowR[0:1, tcols]),
                            rhs=_r(negS[0:1, cols]),
                            start=False, stop=True, skip_group_check=True)
                kpc = kpcp.tile([128, N], F32, tag="kpc")
                nc.scalar.activation(
                    kpc[:, :], kp[:, :], AF.Prelu, bias=negSc[:, t:t + 1],
                    scale=1.0, alpha=1.0)
                state[t] = {'kpc': kpc}

            def emit_and_or(t0, t1):
                # interleave two tiles' chains so back-to-back RAW pipeline
                # bubbles on DVE are hidden
                for t in (t0, t1):
                    kb = keyp.tile([128, N], U32, tag="kb")
                    state[t]['kb'] = kb
                    nc.vector.tensor_scalar(
                        kb[:, :], state[t]['kpc'][:, :].bitcast(U32),
                        0xFFFFFC00, None, op0=ALU.bitwise_and)
                for t in (t0, t1):
                    kb = state[t]['kb']
                    nc.vector.tensor_tensor(
                        out=kb[:, :], in0=kb[:, :], in1=iotaJ[:, :],
                        op=ALU.bitwise_or)

            def emit_topk_pair(t0, t1):
                kbf = {t: state[t]['kb'][:, :].bitcast(F32) for t in (t0, t1)}
                v24 = {}
                for t in (t0, t1):
                    v24[t] = smalls.tile([128, 24], F32, tag="v24", bufs=4, name="v24")
                    nc.vector.max(v24[t][:, 0:8], kbf[t])
                for t in (t0, t1):
                    nc.vector.match_replace(kbf[t], v24[t][:, 0:8], kbf[t], NEG_BIG)
                for t in (t0, t1):
                    nc.vector.max(v24[t][:, 8:16], kbf[t])
                for t in (t0, t1):
                    nc.vector.match_replace(kbf[t], v24[t][:, 8:16], kbf[t], NEG_BIG)
                for t in (t0, t1):
                    nc.vector.max(v24[t][:, 16:24], kbf[t])
                for t in (t0, t1):
                    j20 = smalls.tile([128, KNN], U32, tag="j20")
                    nc.vector.tensor_scalar(
                        j20[:, :], v24[t][:, 0:KNN].bitcast(U32), 0x3FF, None,
                        op0=ALU.bitwise_and)
                    jf20 = smalls.tile([128, KNN], F32, tag="jf20")
                    nc.vector.tensor_copy(jf20[:, :], j20[:, :])
                    state[t]['jf20'] = jf20

            def emit_idq(t):
                jT_ps = pss.tile([KNN, 128], F32, tag="a")
                nc.tensor.matmul(
                    jT_ps[:, :], lhsT=state[t]['jf20'][:, :], rhs=ident[:, 0:128],
                    is_transpose=True, start=True, stop=True, skip_group_check=True)
                jTi = smalls.tile([KNN, 128], I16, tag="jTi")
                nc.scalar.activation(jTi[:, :], jT_ps[:, :], AF.Copy)
                dst = bass.AP(jwap.tensor, jwap.offset + t * 160 * 128,
                              [[1024, KNN], [128, 8], [1, 16]])
                nc.sync.dma_start(
                    dst, jTi[:, :].rearrange("k (h s) -> k h s", s=16))
                src_ap = bass.AP(jwap.tensor, jwap.offset + t * 160 * 128,
                                 [[128, 160], [1, 128]])
                idq = idqp.tile([128, 160], I16, tag="idq")
                nc.sync.dma_start_transpose(idq[:, :], src_ap)
                nc.sync.dma_start(idq[16:32, :], idq[0:16, :])
                nc.sync.dma_start(idq[32:64, :], idq[0:32, :])
                nc.sync.dma_start(idq[64:128, :], idq[0:64, :])
                state[t]['idq'] = idq

            def emit_gathers(t):
                g = gath.tile([128, KNN, O], gdt, tag="g")
                idq = state[t]['idq']
                for q in range(NQ):
                    nc.gpsimd.dma_gather(
                        out_ap=g[:, 5 * q:5 * q + 5, :], in_ap=a_d[:, :],
                        idxs_ap=idq[:, 40 * q:40 * (q + 1)],
                        num_idxs=5 * 128, num_idxs_reg=5 * 128, elem_size=O,
                        single_packet=False, queue_num=q)
                state[t]['g'] = g

            def emit_reduce_pair(t0, t1):
                # contiguous max trees over k=20, two tiles interleaved
                r = {}
                for t in (t0, t1):
                    g = state[t]['g']
                    r[t] = gath.tile([128, 10, O], gdt, tag="r", bufs=2, name="rtree")
                    nc.vector.tensor_tensor(
                        out=r[t][:, :, :], in0=g[:, 0:10, :], in1=g[:, 10:20, :],
                        op=ALU.max)
                for t in (t0, t1):
                    nc.vector.tensor_tensor(
                        out=r[t][:, 0:5, :], in0=r[t][:, 0:5, :],
                        in1=r[t][:, 5:10, :], op=ALU.max)
                for t in (t0, t1):
                    nc.vector.tensor_tensor(
                        out=r[t][:, 0:2, :], in0=r[t][:, 0:2, :],
                        in1=r[t][:, 2:4, :], op=ALU.max)
                for t in (t0, t1):
                    nc.vector.tensor_tensor(
                        out=r[t][:, 0, :], in0=r[t][:, 0, :], in1=r[t][:, 1, :],
                        op=ALU.max)
                for t in (t0, t1):
                    nc.vector.tensor_tensor(
                        out=m[:, t, :], in0=r[t][:, 0, :], in1=r[t][:, 4, :],
                        op=ALU.max)
                for t in (t0, t1):
                    for pi, (dst_t, orow, oc0) in enumerate(out_parts):
                        nc.tensor.matmul(
                            pxs[pi][:, t * 128:(t + 1) * 128],
                            lhsT=m[:, t, oc0:oc0 + orow], rhs=ident[:, 0:128],
                            is_transpose=True, start=False, stop=(t % 4 == 3),
                            skip_group_check=True)
                    state[t] = None  # release refs

            def emit_out_half(h):
                # Prelu with per-partition bias folds the bias matmul; the px
                # bank is finalized by its last transpose (stop at t%4==3)
                cols = slice(h * 512, (h + 1) * 512)
                for pi, (dst_t, orow, oc0) in enumerate(out_parts):
                    nc.scalar.activation(
                        _r(dst_t[0:orow, cols]), pxs[pi][:, cols], AF.Prelu,
                        bias=bs[0:orow, pi:pi + 1], scale=1.0, alpha=NEG_SLOPE)

            # ---- pipelined tile loop, two tiles per round ----
            for p in range(NT // 2):
                t0, t1 = 2 * p, 2 * p + 1
                emit_negSc(t0)
                emit_kp(t0)
                emit_negSc(t1)
                emit_kp(t1)
                if p == 0:
                    emit_a_rows()
                emit_and_or(t0, t1)
                if p == 1:
                    emit_c_parts()
                    if post_tiles is not None:
                        post_tiles()
                if p >= 1:
                    emit_idq(t0 - 2)
                    emit_gathers(t0 - 2)
                    emit_idq(t0 - 1)
                    emit_gathers(t0 - 1)
                emit_topk_pair(t0, t1)
                if p >= 2:
                    emit_reduce_pair(t0 - 4, t1 - 4)
                    if t1 - 4 == 3:
                        emit_out_half(0)
                        if post_half is not None:
                            post_half(0)
            emit_idq(NT - 2)
            emit_gathers(NT - 2)
            emit_idq(NT - 1)
            emit_gathers(NT - 1)
            emit_reduce_pair(NT - 4, NT - 3)
            emit_reduce_pair(NT - 2, NT - 1)
            emit_out_half(1)
            if post_half is not None:
                post_half(1)

        # ---- conv5 weights staged early ----
        w5sb = {}
        for ci, (rows, k0) in enumerate([(64, 0), (64, 64), (128, 128),
                                         (128, 256), (128, 384)]):
            w5c = consts.tile([rows, 512], F32, tag=f"w5c{ci}")
            nc.sync.dma_start(_r(w5c[:, :]), _r(w5t_d[k0:k0 + rows, :]))
            w5sb[ci] = w5c
        b5sb = consts.tile([1, 512], F32, tag="b5sb")
        nc.sync.dma_start(b5sb[:, :], b5_d[:, :])
        zpart = persist.tile([128, NT, 512], F32, tag="zpart")
        zp12 = persist.tile([128, NT, 512], F32, tag="zp12")

        def zp12_fill():
            for t in range(NT):
                tcols = slice(t * 128, (t + 1) * 128)
                zp_ps = pss.tile([128, 512], F32, tag="a")
                for ci, (xt, rows) in enumerate([(x1T, 64), (x2T, 64)]):
                    nc.tensor.matmul(
                        zp_ps[:, :], lhsT=_r(xt[0:rows, tcols]),
                        rhs=_r(w5sb[ci][:, :]),
                        start=(ci == 0), stop=(ci == 1), skip_group_check=True)
                nc.scalar.activation(_r(zp12[:, t, :]), zp_ps[:, :], AF.Copy)

        def zpart_fill():
            for t in range(NT):
                tcols = slice(t * 128, (t + 1) * 128)
                zp_ps = pss.tile([128, 512], F32, tag="a")
                nc.tensor.matmul(
                    zp_ps[:, :], lhsT=_r(x3T[:, tcols]), rhs=_r(w5sb[2][:, :]),
                    start=True, stop=False, skip_group_check=True)
                nc.tensor.matmul(
                    zp_ps[:, :], lhsT=ident[:, 0:128], rhs=zp12[:, t, :],
                    start=False, stop=True, skip_group_check=True)
                nc.scalar.activation(zpart[:, t, :], zp_ps[:, :], AF.Copy)

        edge_layer(1, x0T, 3, 64, [(x1T, 64, 0)])
        edge_layer(2, x1T, 64, 64, [(x2T, 64, 0)])
        edge_layer(3, x2T, 64, 128, [(x3T, 128, 0)], post_tiles=zp12_fill)
        edge_layer(4, x3T, 128, 256, [(x4Ta, 128, 0), (x4Tb, 128, 128)],
                   post_tiles=zpart_fill)

        # ---- head: conv5 (x4 chunks; x1-x3 partials precomputed) + max pool ----
        zmax = persist.tile([128, 512], F32, tag="zmax")
        for t in range(NT):
            tcols = slice(t * 128, (t + 1) * 128)
            z_ps = pss.tile([128, 512], F32, tag="a")
            for ci, (xt, k0) in enumerate([(x4Ta, 256), (x4Tb, 384)]):
                nc.tensor.matmul(
                    z_ps[:, :], lhsT=_r(xt[:, tcols]), rhs=_r(w5sb[3 + ci][:, :]),
                    start=(ci == 0), stop=False, skip_group_check=True)
            nc.tensor.matmul(
                z_ps[:, :], lhsT=onesrow[0:1, tcols],
                rhs=b5sb[:, :], start=False, stop=True, skip_group_check=True)
            zsb = sb.tile([128, 512], F32, tag="zsb")
            nc.vector.tensor_tensor(
                out=zsb[:, :], in0=zpart[:, t, :], in1=z_ps[:, :], op=ALU.add)
            if t == 0:
                nc.scalar.activation(zmax[:, :], zsb[:, :], AF.Copy)
            else:
                nc.vector.tensor_tensor(
                    out=zmax[:, :], in0=zmax[:, :], in1=zsb[:, :], op=ALU.max)
        # transpose zmax chunks and reduce along free dim -> yT [128, 4]
        yT = persist.tile([128, 4], F32, tag="yT")
        for cchunk in range(4):
            zt_ps = pss.tile([128, 128], F32, tag="a")
            nc.tensor.matmul(
                zt_ps[:, :], lhsT=zmax[:, cchunk * 128:(cchunk + 1) * 128],
                rhs=ident[:, 0:128], is_transpose=True, start=True, stop=True,
                skip_group_check=True)
            nc.vector.tensor_reduce(
                out=yT[:, cchunk:cchunk + 1], in_=zt_ps[:, :],
                axis=AX.X, op=ALU.max)
        yTr = persist.tile([128, 4], F32, tag="yTr")
        nc.scalar.activation(yTr[:, :], yT[:, :], AF.Prelu, alpha=NEG_SLOPE)

        # ---- FC head ----
        wfc1sb = consts.tile([128, 4, 256], F32, tag="wfc1sb")
        for c in range(4):
            nc.sync.dma_start(wfc1sb[:, c, :], wfc1_d[c * 128:(c + 1) * 128, :])
        bfc1sb = consts.tile([128, 2], F32, tag="bfc1sb")
        nc.sync.dma_start(bfc1sb[:, :], bfc1_d[:, :])
        wfc2sb = consts.tile([128, 2, 128], F32, tag="wfc2sb")
        for c in range(2):
            nc.sync.dma_start(wfc2sb[:, c, :], wfc2_d[c * 128:(c + 1) * 128, :])
        bfc2sb = consts.tile([128, 1], F32, tag="bfc2sb")
        nc.sync.dma_start(bfc2sb[:, :], bfc2_d[:, :])
        wfc3sb = consts.tile([128, 40], F32, tag="wfc3sb")
        nc.sync.dma_start(wfc3sb[:, :], wfc3_d[:, :])
        bfc3sb = consts.tile([1, 40], F32, tag="bfc3sb")
        nc.sync.dma_start(bfc3sb[:, :], bfc3_d[:, :])

        h1sb = persist.tile([128, 2], F32, tag="h1sb")
        for mt in range(2):
            h1_ps = pss.tile([128, 1], F32, tag="a")
            for c in range(4):
                nc.tensor.matmul(
                    h1_ps[:, :], lhsT=wfc1sb[:, c, mt * 128:(mt + 1) * 128],
                    rhs=yTr[:, c:c + 1],
                    start=(c == 0), stop=(c == 3), skip_group_check=True)
            nc.scalar.activation(
                h1sb[:, mt:mt + 1], h1_ps[:, :], AF.Prelu,
                bias=bfc1sb[:, mt:mt + 1], scale=1.0, alpha=NEG_SLOPE)
        h2sb = persist.tile([128, 1], F32, tag="h2sb")
        h2_ps = pss.tile([128, 1], F32, tag="a")
        for c in range(2):
            nc.tensor.matmul(
                h2_ps[:, :], lhsT=wfc2sb[:, c, :], rhs=h1sb[:, c:c + 1],
                start=(c == 0), stop=(c == 1), skip_group_check=True)
        nc.scalar.activation(
            h2sb[:, :], h2_ps[:, :], AF.Prelu,
            bias=bfc2sb[:, :], scale=1.0, alpha=NEG_SLOPE)

        out_ps = pss.tile([40, 1], F32, tag="a")
        nc.tensor.matmul(
            out_ps[:, :], lhsT=wfc3sb[:, :], rhs=h2sb[:, :],
            start=True, stop=False, skip_group_check=True)
        nc.tensor.matmul(
            out_ps[:, :], lhsT=bfc3sb[:, :], rhs=onescol[0:1, :],
            start=False, stop=True, skip_group_check=True)
        out_sb = persist.tile([40, 1], F32, tag="out_sb")
        nc.scalar.activation(out_sb[:, :], out_ps[:, :], AF.Copy)
        nc.sync.dma_start(out_d[:, :], out_sb[:, :])


# ---------------------------------------------------------------------------
# harness entry point
# ---------------------------------------------------------------------------
_NC_CACHE = {}


def _get_nc():
    if 'nc' not in _NC_CACHE:
        _NC_CACHE['nc'] = build_nc()
    return _NC_CACHE['nc']


def kernel(**inputs):
    """Full-batch EdgeCNN forward. x: (8, 1024, 3) -> (8, 40) float32.

    Pure data parallel: batch element b runs on NeuronCore b.
    """
    from concourse.bass_utils import run_bass_kernel_spmd

    inp = {k: np.asarray(v) for k, v in inputs.items()}
    prep = host_prep(inp)
    nc = _get_nc()
    in_maps = []
    for b in range(8):
        m = {'x': np.ascontiguousarray(inp['x'][b]).astype(np.float32)}
        m.update(prep)
        in_maps.append(m)
    res = run_bass_kernel_spmd(nc, in_maps, core_ids=list(range(8)))
    out = np.stack([res.results[b]['out'].reshape(40) for b in range(8)])
    return out.astype(np.float32)
